# revision 1
# baseline (speedup 1.0000x reference)
"""Trainium2 Bass kernel for nn_BDH_6313601925221 (sparse_attention).

Model (reference.py):
  x = LN(embed[idx])                                   (B=1, T=1024, D=256)
  repeat 6 layers (shared weights):
    x_sparse = relu(einsum('btd,hdn->bhtn', x, encoder))   N=8192, NH=4
    QR       = rope(x_sparse)                              interleaved-pair rotation
    scores   = einsum('bhtn,bhsn->bhts', QR, QR) * strict_causal
    yKV      = LN(einsum('bhts,bsd->bhtd', scores, x))
    y_sparse = relu(einsum('bhtd,hdn->bhtn', yKV, encoder_v))
    yMLP     = (x_sparse*y_sparse).transpose -> (T, NH*N) @ decoder
    x        = LN(x + LN(yMLP))
  logits = x @ lm_head

Distribution (8 cores): core c = (head h=c//2, latent-half eta=c%2).
Each core computes the encoder/rope/scores path over its 4096 latent dims
(pairwise AllReduce of partial scores within the head pair), duplicates the
small yKV path, then computes y_sparse/xy/decoder over its latent half for
all tokens; one 8-rank AllReduce of the yMLP partials per layer.

Layouts: latent dim N is host-permuted so rope pairs are de-interleaved:
local tile 2j = even pair members, 2j+1 = odd. Inner products over N and
the decoder contraction are invariant to this permutation (weights are
permuted to match).

PSUM budget (8 banks): acc_a/acc_b/acc_c [128,1024] f32 (2 banks each,
bufs=1) carry all long-lived accumulations (score strips, yKV, yMLP);
ps_w [128,512] (bufs=2) carries transient matmul outputs.
"""

import math
import sys

import numpy as np

for _p in ("/opt/trn_rl_repo",):
    if _p not in sys.path:
        sys.path.insert(0, _p)

import concourse.bass as bass
import concourse.mybir as mybir
import concourse.tile as tile
from concourse import bacc
from concourse import bass_utils

# ---------------------------------------------------------------- constants
D = 256
NH = 4
N = 8192
T = 1024
N_LAYER = 6
VOCAB = 256
THETA = 2 ** 16
EPS = 1e-5
NCORES = 8

NHALF = N // 2          # 4096 latent dims per core
NPAIR = NHALF // 2      # 2048 rope pairs per core
NT = NHALF // 128       # 32 local n-tiles of 128
NJ = NT // 2            # 16 pair-blocks (tile 2j = evens, 2j+1 = odds)
TB = T // 128           # 8 token blocks
DC = D // 128           # 2 d-chunks

F16 = mybir.dt.float16
F32 = mybir.dt.float32
I32 = mybir.dt.int32
AX = mybir.AxisListType
ALU = mybir.AluOpType
ACTF = mybir.ActivationFunctionType

# kb -> (group, acc tag, column offset inside the [128,1024] acc tile)
SC_LAYOUT = {
    0: (0, "acc_a", 0),
    1: (0, "acc_b", 0),
    2: (0, "acc_c", 0),
    3: (1, "acc_a", 0),
    4: (1, "acc_b", 0),
    5: (1, "acc_b", 512),
    6: (1, "acc_c", 0),
    7: (1, "acc_c", 512),
}


def _bi(kb, qb):
    """Linear index of score block (kb, qb), kb <= qb."""
    return kb * TB - (kb * (kb - 1)) // 2 + (qb - kb)


def _ln_free(nc, pool, x_ap, eps_ap, out_f32=None, out_f16=None,
             skip_mean=False, n=None, name=""):
    """LayerNorm along the free dim of a [128, n] tile (per-partition stats)."""
    n = n if n is not None else x_ap.shape[-1]
    inv_n = 1.0 / n
    sq = pool.tile([128, n], F32, name=f"lnsq{name}", tag="lnsq")
    ssq = pool.tile([128, 1], F32, name=f"lnssq{name}", tag="lnssq")
    std = pool.tile([128, 1], F32, name=f"lnstd{name}", tag="lnstd")
    inv = pool.tile([128, 1], F32, name=f"lninv{name}", tag="lninv")
    if skip_mean:
        xm = x_ap
    else:
        mu = pool.tile([128, 1], F32, name=f"lnmu{name}", tag="lnmu")
        xm_t = pool.tile([128, n], F32, name=f"lnxm{name}", tag="lnxm")
        nc.vector.tensor_reduce(mu[:], x_ap, axis=AX.X, op=ALU.add)
        nc.scalar.mul(mu[:], mu[:], inv_n)
        nc.vector.tensor_scalar_sub(xm_t[:], x_ap, mu[:])
        xm = xm_t[:]
    nc.scalar.activation(sq[:], xm, ACTF.Square, accum_out=ssq[:])
    nc.scalar.activation(std[:], ssq[:], ACTF.Sqrt, bias=eps_ap, scale=inv_n)
    nc.vector.reciprocal(inv[:], std[:])
    if out_f32 is not None:
        nc.vector.tensor_scalar_mul(out_f32, xm, inv[:])
    if out_f16 is not None:
        nc.scalar.activation(out_f16, xm, ACTF.Copy, scale=inv[:])
    return xm, inv


def build_program(dbg=False, n_layer=N_LAYER, sim_single=False,
                  stub_sc_ar=False, stub_ym_ar=False, tiny_ar=False):
    if sim_single:
        stub_sc_ar = stub_ym_ar = True
    nc = bacc.Bacc("TRN2", target_bir_lowering=False, debug=False,
                   num_devices=NCORES)
    dbg_o = {}
    if dbg:
        dbg_o["x0"] = nc.dram_tensor("dbg_x0", [T, D], F32, kind="ExternalOutput")
        dbg_o["xs"] = nc.dram_tensor("dbg_xs", [256, T], F32, kind="ExternalOutput")
        dbg_o["qr"] = nc.dram_tensor("dbg_qr", [256, T], F32, kind="ExternalOutput")
        dbg_o["st"] = nc.dram_tensor("dbg_st", [36 * 128, 128], F32, kind="ExternalOutput")
        dbg_o["ykv"] = nc.dram_tensor("dbg_ykv", [T, D], F32, kind="ExternalOutput")
        dbg_o["ym"] = nc.dram_tensor("dbg_ym", [T, D], F32, kind="ExternalOutput")
        dbg_o["x1"] = nc.dram_tensor("dbg_x1", [T, D], F32, kind="ExternalOutput")
        dbg_o["ymp"] = nc.dram_tensor("dbg_ymp", [D, T], F16, kind="ExternalOutput")
        dbg_o["ykvT"] = nc.dram_tensor("dbg_ykvT", [256, T], F32, kind="ExternalOutput")

    # ------------------------------------------------------------- I/O decl
    idx_i = nc.dram_tensor("idx32", [T, 1], F32, kind="ExternalInput")
    embed_i = nc.dram_tensor("embed", [VOCAB, D], F32, kind="ExternalInput")
    enc_i = nc.dram_tensor("enc_sh", [D, NHALF], F16, kind="ExternalInput")
    encv_i = nc.dram_tensor("encv_sh", [D, NHALF], F16, kind="ExternalInput")
    dec_i = nc.dram_tensor("dec_sh", [NHALF, D], F16, kind="ExternalInput")
    lmh_i = nc.dram_tensor("lmh", [D, VOCAB], F16, kind="ExternalInput")
    cos_i = nc.dram_tensor("cos_sh", [NPAIR, T], F16, kind="ExternalInput")
    sin_i = nc.dram_tensor("sin_sh", [NPAIR, T], F16, kind="ExternalInput")
    cmask_i = nc.dram_tensor("cmask", [128, 128], F16, kind="ExternalInput")
    ident_i = nc.dram_tensor("ident", [128, 128], F16, kind="ExternalInput")
    ident32_i = nc.dram_tensor("ident32", [128, 128], F32, kind="ExternalInput")
    out_o = nc.dram_tensor("logits", [T, VOCAB], F32, kind="ExternalOutput")

    pair_groups = [[2 * h, 2 * h + 1] for h in range(NH)]
    all_group = [list(range(NCORES))]

    with tile.TileContext(nc) as tc:
      with (
        tc.tile_pool(name="persist", bufs=1) as pp,
        tc.tile_pool(name="work", bufs=2) as wp,
        tc.tile_pool(name="psW", bufs=2, space="PSUM") as psW,
        tc.tile_pool(name="psAcc", bufs=1, space="PSUM") as psAcc,
        tc.tile_pool(name="dram", bufs=1, space="DRAM") as dp,
      ):
        # ------------------------------------------------- persistent SBUF
        enc_sb = [pp.tile([128, NHALF], F16, name=f"enc{d}", tag=f"enc{d}")
                  for d in range(DC)]
        encv_sb = [pp.tile([128, NHALF], F16, name=f"encv{d}", tag=f"encv{d}")
                   for d in range(DC)]
        QR = [pp.tile([128, T], F16, name=f"qr{i}", tag=f"qr{i}")
              for i in range(NT)]
        ST = [pp.tile([128, 128], F16, name=f"st{i}", tag=f"st{i}")
              for i in range(36)]  # S^T blocks (kb,qb) kb<=qb, fp16, masked
        x_t32 = [pp.tile([128, D], F32, name=f"xt32_{i}", tag=f"xt32_{i}")
                 for i in range(TB)]
        x_t16 = [pp.tile([128, D], F16, name=f"xt16_{i}", tag=f"xt16_{i}")
                 for i in range(TB)]
        x_d16 = [pp.tile([128, T], F16, name=f"xd16_{i}", tag=f"xd16_{i}")
                 for i in range(DC)]
        ykv_t = [pp.tile([128, D], F16, name=f"ykvt{i}", tag=f"ykvt{i}")
                 for i in range(TB)]
        ykvT = [pp.tile([128, T], F16, name=f"ykvT{i}", tag=f"ykvT{i}")
                for i in range(DC)]
        cmask = pp.tile([128, 128], F16, name="cmaskt", tag="cmaskt")
        eps_t = pp.tile([128, 1], F32, name="eps_t", tag="eps_t")
        ident = pp.tile([128, 128], F16, name="identt", tag="identt")
        ident32 = pp.tile([128, 128], F32, name="identt32", tag="identt32")
        lmh_sb = [pp.tile([128, VOCAB], F16, name=f"lmh{d}", tag=f"lmh{d}")
                  for d in range(DC)]

        # ---------------------------------------------------- DRAM buffers
        xs_spill = dp.tile([NHALF, T], F16, name="xs_spill")
        sc_in0 = dp.tile([21 * 128, 128], F16, name="sc_in0")
        sc_out0 = dp.tile([21 * 128, 128], F16, name="sc_out0")
        sc_in1 = dp.tile([15 * 128, 128], F16, name="sc_in1")
        sc_out1 = dp.tile([15 * 128, 128], F16, name="sc_out1")
        tin = dp.tile([128, 128], F16, name="tin")
        touts = [dp.tile([128, 128], F16, name=f"tout{l}", tag=f"tout{l}")
                 for l in range(n_layer)]
        touts8 = [dp.tile([128, 128], F16, name=f"tout8{l}", tag=f"tout8{l}",
                  addr_space="Shared") for l in range(n_layer)]
        ym_in = dp.tile([D, T], F16, name="ym_in")
        ym_outs = [dp.tile([D, T], F16, name=f"ym_out{l}", tag=f"ym_out{l}",
                           addr_space="Shared") for l in range(n_layer)]

        def psw(name, shape=(128, 512), dtype=F32):
            return psW.tile(list(shape), dtype, name=name, tag="ps_w",
                            padded_shape=[128, 512])

        def dbg_dump16(dst_dram, row0, src_ap, w):
            tt = wp.tile([128, w], F32, name="dbgt", tag="dbgt", bufs=1)
            nc.vector.tensor_copy(tt[:], src_ap)
            nc.sync.dma_start(dst_dram[row0:row0 + 128, :], tt[:])

        # ------------------------------------------------------ load consts
        nc.gpsimd.memset(eps_t[:], EPS)
        nc.sync.dma_start(cmask[:], cmask_i[:, :])
        nc.sync.dma_start(ident[:], ident_i[:, :])
        nc.sync.dma_start(ident32[:], ident32_i[:, :])
        for d in range(DC):
            nc.sync.dma_start(enc_sb[d][:], enc_i[128 * d:128 * (d + 1), :])
            nc.sync.dma_start(encv_sb[d][:], encv_i[128 * d:128 * (d + 1), :])
            nc.sync.dma_start(lmh_sb[d][:], lmh_i[128 * d:128 * (d + 1), :])

        # ------------------------------------------------------- embedding
        # E_n = LN(embed) per vocab row; x0 = onehot(idx) @ E_n
        with tc.tile_pool(name="embed", bufs=1) as ep:
            E_n = [ep.tile([128, D], F16, name=f"en{v}", tag=f"en{v}")
                   for v in range(DC)]
            for v in range(DC):
                emb_raw = ep.tile([128, D], F32, name=f"emb_raw{v}",
                                  tag=f"emb_raw{v}")
                nc.sync.dma_start(emb_raw[:], embed_i[128 * v:128 * (v + 1), :])
                _ln_free(nc, wp, emb_raw[:], eps_t[:], out_f16=E_n[v][:],
                         name=f"emb{v}")

            iota_i32 = ep.tile([128, VOCAB], I32, name="iota_i32",
                               tag="iota_i32")
            nc.gpsimd.iota(iota_i32[:], pattern=[[1, VOCAB]], base=0,
                           channel_multiplier=0)
            iota_t = ep.tile([128, VOCAB], F32, name="iota_t", tag="iota_t")
            nc.vector.tensor_copy(iota_t[:], iota_i32[:])
            OHT = [ep.tile([128, T], F16, name=f"oht{v}", tag=f"oht{v}")
                   for v in range(DC)]
            for tb in range(TB):
                idx_col = wp.tile([128, 1], F32, name="idx_col", tag="idx_col")
                nc.sync.dma_start(idx_col[:], idx_i[128 * tb:128 * (tb + 1), :])
                oh_tm = wp.tile([128, VOCAB], F16, name="oh_tm", tag="oh_tm")
                nc.vector.tensor_scalar(oh_tm[:], iota_t[:], idx_col[:], None,
                                        op0=ALU.is_equal)
                for v in range(DC):
                    ps_t = psw(f"ps_tr_oh{tb}_{v}", (128, 128), F16)
                    nc.tensor.transpose(ps_t[:],
                                        oh_tm[:, 128 * v:128 * (v + 1)],
                                        ident[:])
                    nc.scalar.copy(OHT[v][:, 128 * tb:128 * (tb + 1)], ps_t[:])

            for tb in range(TB):
                ps_x = psw(f"ps_x0_{tb}", (128, D))
                for v in range(DC):
                    nc.tensor.matmul(ps_x[:],
                                     OHT[v][:, 128 * tb:128 * (tb + 1)],
                                     E_n[v][:], start=(v == 0),
                                     stop=(v == DC - 1))
                nc.vector.tensor_copy(x_t32[tb][:], ps_x[:])
                nc.scalar.copy(x_t16[tb][:], ps_x[:])
            for d in range(DC):
                for th in range(2):
                    ps_xd = psw(f"ps_xd_{d}_{th}")
                    for v in range(DC):
                        nc.tensor.matmul(
                            ps_xd[:], E_n[v][:, 128 * d:128 * (d + 1)],
                            OHT[v][:, 512 * th:512 * (th + 1)],
                            start=(v == 0), stop=(v == DC - 1))
                    nc.scalar.copy(x_d16[d][:, 512 * th:512 * (th + 1)],
                                   ps_xd[:])

        if dbg:
            for tb in range(TB):
                dbg_dump16(dbg_o["x0"], 128 * tb, x_t32[tb][:], D)

        # ============================================================ layers
        for layer in range(n_layer):
            # ---------------- phase 1a: x_sparse + rope + scores group 0
            acc = {t: psAcc.tile([128, 1024], F32, name=f"{t}_s0_{layer}",
                                 tag=t) for t in ("acc_a", "acc_b", "acc_c")}

            def sc_ap(kb, grp_acc):
                _, tag, off = SC_LAYOUT[kb]
                w = (TB - kb) * 128
                return grp_acc[tag][:, off:off + w]

            for j in range(NJ):
                ct = wp.tile([128, T], F16, name="cos_t", tag="cos_t")
                st_t = wp.tile([128, T], F16, name="sin_t", tag="sin_t")
                nc.sync.dma_start(ct[:], cos_i[128 * j:128 * (j + 1), :])
                nc.sync.dma_start(st_t[:], sin_i[128 * j:128 * (j + 1), :])
                xs_pair = []
                for par in range(2):  # even tile, odd tile
                    nt = 2 * j + par
                    xs_sb = wp.tile([128, T], F16, name="xs_sb", tag="xs_sb")
                    for th in range(2):
                        ps_e = psw(f"ps_enc_{layer}_{nt}_{th}")
                        for d in range(DC):
                            nc.tensor.matmul(
                                ps_e[:],
                                enc_sb[d][:, 128 * nt:128 * (nt + 1)],
                                x_d16[d][:, 512 * th:512 * (th + 1)],
                                start=(d == 0), stop=(d == DC - 1))
                        nc.scalar.activation(xs_sb[:, 512 * th:512 * (th + 1)],
                                             ps_e[:], ACTF.Relu)
                    nc.sync.dma_start(
                        xs_spill[128 * nt:128 * (nt + 1), :], xs_sb[:])
                    xs_pair.append(xs_sb)
                # rope: qr_e = xs_e*c - xs_o*s ; qr_o = xs_o*c + xs_e*s
                xe, xo = xs_pair[0], xs_pair[1]
                qe, qo = QR[2 * j], QR[2 * j + 1]
                p1 = wp.tile([128, T], F16, name="rp1", tag="rp1")
                p2 = wp.tile([128, T], F16, name="rp2", tag="rp2")
                nc.vector.tensor_mul(p1[:], xe[:], ct[:])
                nc.gpsimd.tensor_mul(p2[:], xo[:], st_t[:])
                nc.vector.tensor_sub(qe[:], p1[:], p2[:])
                nc.vector.tensor_mul(p1[:], xo[:], ct[:])
                nc.gpsimd.tensor_mul(p2[:], xe[:], st_t[:])
                nc.vector.tensor_add(qo[:], p1[:], p2[:])
                if dbg and layer == 0 and j == 0:
                    dbg_dump16(dbg_o["xs"], 0, xe[:], T)
                    dbg_dump16(dbg_o["xs"], 128, xo[:], T)
                    dbg_dump16(dbg_o["qr"], 0, qe[:], T)
                    dbg_dump16(dbg_o["qr"], 128, qo[:], T)
                # scores group-0 accumulation for these two n-chunks
                for par in range(2):
                    nt = 2 * j + par
                    first = (j == 0 and par == 0)
                    last = (j == NJ - 1 and par == 1)
                    for kb in range(TB):
                        if SC_LAYOUT[kb][0] != 0:
                            continue
                        dst = sc_ap(kb, acc)
                        w = (TB - kb) * 128
                        for nn in range(0, w, 512):
                            nw = min(512, w - nn)
                            nc.tensor.matmul(
                                dst[:, nn:nn + nw],
                                QR[nt][:, 128 * kb:128 * (kb + 1)],
                                QR[nt][:, 128 * kb + nn:128 * kb + nn + nw],
                                start=first, stop=last)
            # spill score group 0 to DRAM bounce (fp16 via SBUF)
            for kb in range(TB):
                if SC_LAYOUT[kb][0] != 0:
                    continue
                src = sc_ap(kb, acc)
                for qb in range(kb, TB):
                    s_sb = wp.tile([128, 128], F16, name="s_sb", tag="s_sb")
                    nc.scalar.copy(
                        s_sb[:],
                        src[:, 128 * (qb - kb):128 * (qb - kb + 1)])
                    nc.sync.dma_start(
                        sc_in0[128 * _bi(kb, qb):128 * (_bi(kb, qb) + 1), :],
                        s_sb[:])
            # AR of group 0 overlaps with the group-1 matmuls below
            if stub_sc_ar:
                nc.sync.dma_start(sc_out0[:, :], sc_in0[:, :])
                if tiny_ar:
                    nc.gpsimd.collective_compute(
                        "AllReduce", ALU.add, replica_groups=pair_groups,
                        ins=[tin.opt()], outs=[touts[layer].opt()])
            else:
                nc.gpsimd.collective_compute(
                    "AllReduce", ALU.add, replica_groups=pair_groups,
                    ins=[sc_in0.opt()], outs=[sc_out0.opt()])
            for kb in range(TB):
                if SC_LAYOUT[kb][0] != 0:
                    continue
                for qb in range(kb, TB):
                    blk = ST[_bi(kb, qb)]
                    nc.sync.dma_start(
                        blk[:],
                        sc_out0[128 * _bi(kb, qb):128 * (_bi(kb, qb) + 1), :])
                    if qb == kb:
                        nc.vector.tensor_mul(blk[:], blk[:], cmask[:])
            # ---------------- phase 1b: scores group 1 (QR resident)
            acc1 = {t: psAcc.tile([128, 1024], F32, name=f"{t}_s1_{layer}",
                                  tag=t) for t in ("acc_a", "acc_b", "acc_c")}
            for nt in range(NT):
                for kb in range(TB):
                    if SC_LAYOUT[kb][0] != 1:
                        continue
                    dst = sc_ap(kb, acc1)
                    w = (TB - kb) * 128
                    for nn in range(0, w, 512):
                        nw = min(512, w - nn)
                        nc.tensor.matmul(
                            dst[:, nn:nn + nw],
                            QR[nt][:, 128 * kb:128 * (kb + 1)],
                            QR[nt][:, 128 * kb + nn:128 * kb + nn + nw],
                            start=(nt == 0), stop=(nt == NT - 1))
            for kb in range(TB):
                if SC_LAYOUT[kb][0] != 1:
                    continue
                src = sc_ap(kb, acc1)
                for qb in range(kb, TB):
                    s_sb = wp.tile([128, 128], F16, name="s_sb", tag="s_sb")
                    nc.scalar.copy(
                        s_sb[:],
                        src[:, 128 * (qb - kb):128 * (qb - kb + 1)])
                    nc.sync.dma_start(
                        sc_in1[128 * (_bi(kb, qb) - 21):
                               128 * (_bi(kb, qb) - 20), :],
                        s_sb[:])
            # ---------------- scores AllReduce (group 1)
            if stub_sc_ar:
                nc.sync.dma_start(sc_out1[:, :], sc_in1[:, :])
            else:
                nc.gpsimd.collective_compute(
                    "AllReduce", ALU.add, replica_groups=pair_groups,
                    ins=[sc_in1.opt()], outs=[sc_out1.opt()])
            for kb in range(TB):
                if SC_LAYOUT[kb][0] != 1:
                    continue
                for qb in range(kb, TB):
                    blk = ST[_bi(kb, qb)]
                    nc.sync.dma_start(
                        blk[:],
                        sc_out1[128 * (_bi(kb, qb) - 21):
                                128 * (_bi(kb, qb) - 20), :])
                    if qb == kb:
                        nc.vector.tensor_mul(blk[:], blk[:], cmask[:])
            if dbg and layer == 0:
                for kb in range(TB):
                    for qb in range(kb, TB):
                        dbg_dump16(dbg_o["st"], 128 * _bi(kb, qb),
                                   ST[_bi(kb, qb)][:], 128)
            # ---------------- phase 2: yKV + LN + transpose
            # each of the 4 concurrent streams gets its own PSUM bank
            ykv_acc = {}
            for half in range(2):
                for ti, t in enumerate(("acc_a", "acc_b")):
                    ykv_acc[(half, ti)] = psAcc.tile(
                        [128, 1024], F32, name=f"{t}_ykv_{layer}_{half}",
                        tag=t)
            for qb in range(TB):
                ps_y = ykv_acc[(qb // 4, (qb % 4) // 2)][
                    :, 512 * (qb % 2):512 * (qb % 2) + D]
                for kb in range(qb + 1):
                    nc.tensor.matmul(ps_y, ST[_bi(kb, qb)][:], x_t16[kb][:],
                                     start=(kb == 0), stop=(kb == qb))
                _ln_free(nc, wp, ps_y, eps_t[:], out_f16=ykv_t[qb][:],
                         name=f"ykv{qb}")
                for d in range(DC):
                    ps_t = psw(f"ps_tr_ykv{qb}_{d}", (128, 128), F16)
                    nc.tensor.transpose(
                        ps_t[:], ykv_t[qb][:, 128 * d:128 * (d + 1)], ident[:])
                    nc.scalar.copy(ykvT[d][:, 128 * qb:128 * (qb + 1)],
                                   ps_t[:])
            if dbg and layer == 0:
                for qb in range(TB):
                    dbg_dump16(dbg_o["ykv"], 128 * qb, ykv_t[qb][:], D)
                for d in range(DC):
                    dbg_dump16(dbg_o["ykvT"], 128 * d, ykvT[d][:], T)
            # ---------------- phase 3: y_sparse, xy, decoder partials
            # yMLP^T partials [d, t]: one d-half per acc tile; the two
            # 512-wide t-chunks are separate streams in separate banks
            ym_acc = {}
            for half in range(2):
                t = ("acc_a", "acc_b")[half]
                ym_acc[half] = psAcc.tile([128, 1024], F32,
                                          name=f"{t}_ym_{layer}", tag=t)
            for nt in range(NT):
                dec_t = wp.tile([128, D], F16, name="dec_t", tag="dec_t")
                nc.sync.dma_start(dec_t[:], dec_i[128 * nt:128 * (nt + 1), :])
                xs_sb = wp.tile([128, T], F16, name="xs_sb2", tag="xs_sb2")
                nc.sync.dma_start(xs_sb[:],
                                  xs_spill[128 * nt:128 * (nt + 1), :])
                xy = wp.tile([128, T], F16, name="xy", tag="xy")
                for th in range(2):
                    ps_v = psw(f"ps_ysp_{layer}_{nt}_{th}")
                    for d in range(DC):
                        nc.tensor.matmul(
                            ps_v[:], encv_sb[d][:, 128 * nt:128 * (nt + 1)],
                            ykvT[d][:, 512 * th:512 * (th + 1)],
                            start=(d == 0), stop=(d == DC - 1))
                    # xy = relu(ys) * xs  (fused)
                    nc.vector.scalar_tensor_tensor(
                        xy[:, 512 * th:512 * (th + 1)], ps_v[:], 0.0,
                        xs_sb[:, 512 * th:512 * (th + 1)],
                        op0=ALU.max, op1=ALU.mult)
                for dh in range(DC):
                    for thc in range(2):
                        nc.tensor.matmul(
                            ym_acc[dh][:, 512 * thc:512 * (thc + 1)],
                            dec_t[:, 128 * dh:128 * (dh + 1)],
                            xy[:, 512 * thc:512 * (thc + 1)],
                            start=(nt == 0), stop=(nt == NT - 1))
            # ---------------- yMLP AllReduce (sum over heads & halves)
            for dh in range(DC):
                ym_sb = wp.tile([128, T], F16, name="ym_sb", tag="ym_sb",
                                bufs=1)
                nc.vector.tensor_copy(ym_sb[:], ym_acc[dh][:])
                nc.sync.dma_start(ym_in[128 * dh:128 * (dh + 1), :], ym_sb[:])
                if dbg and layer == 0:
                    nc.sync.dma_start(dbg_o["ymp"][128 * dh:128 * (dh + 1), :],
                                      ym_sb[:])
            ym_out = ym_outs[layer]
            if stub_ym_ar:
                nc.sync.dma_start(ym_out[:, :], ym_in[:, :])
                if tiny_ar:
                    nc.gpsimd.collective_compute(
                        "AllReduce", ALU.add, replica_groups=all_group,
                        ins=[tin.opt()], outs=[touts8[layer].opt()])
            else:
                nc.gpsimd.collective_compute(
                    "AllReduce", ALU.add, replica_groups=all_group,
                    ins=[ym_in.opt()], outs=[ym_out.opt()])
            # ---------------- tail: x = LN(x + LN(yMLP))
            um_d = [wp.tile([128, T], F16, name=f"um_d{dh}", tag=f"um_d{dh}",
                            bufs=1)
                    for dh in range(DC)]
            for dh in range(DC):
                nc.sync.dma_start(um_d[dh][:],
                                  ym_out[128 * dh:128 * (dh + 1), :])
            for tb in range(TB):
                u = wp.tile([128, D], F32, name="u_t", tag="u_t")
                for dh in range(DC):
                    ps_t16 = psw(f"ps_tru_{layer}_{tb}_{dh}", (128, 128), F16)
                    nc.tensor.transpose(
                        ps_t16[:], um_d[dh][:, 128 * tb:128 * (tb + 1)],
                        ident[:])
                    nc.scalar.copy(u[:, 128 * dh:128 * (dh + 1)], ps_t16[:])
                if dbg and layer == 0:
                    dbg_dump16(dbg_o["ym"], 128 * tb, u[:], D)
                xm_u, inv_u = _ln_free(nc, wp, u[:], eps_t[:], name=f"u{tb}")
                v = wp.tile([128, D], F32, name="v_t", tag="v_t")
                nc.vector.scalar_tensor_tensor(
                    v[:], xm_u, inv_u[:], x_t32[tb][:],
                    op0=ALU.mult, op1=ALU.add)
                _ln_free(nc, wp, v[:], eps_t[:], out_f32=x_t32[tb][:],
                         out_f16=x_t16[tb][:], skip_mean=True, name=f"v{tb}")
                if dbg and layer == 0:
                    dbg_dump16(dbg_o["x1"], 128 * tb, x_t32[tb][:], D)
                for d in range(DC):
                    ps_t = psw(f"ps_tr_x{layer}_{tb}_{d}", (128, 128), F16)
                    nc.tensor.transpose(
                        ps_t[:], x_t16[tb][:, 128 * d:128 * (d + 1)], ident[:])
                    nc.scalar.copy(x_d16[d][:, 128 * tb:128 * (tb + 1)],
                                   ps_t[:])

        # ------------------------------------------------------- lm head
        for tb in range(TB):
            ps_l = psw(f"ps_lg_{tb}", (128, VOCAB))
            for d in range(DC):
                nc.tensor.matmul(ps_l[:], x_d16[d][:, 128 * tb:128 * (tb + 1)],
                                 lmh_sb[d][:], start=(d == 0),
                                 stop=(d == DC - 1))
            lg_sb = wp.tile([128, VOCAB], F32, name="lg_sb", tag="lg_sb")
            nc.vector.tensor_copy(lg_sb[:], ps_l[:])
            nc.sync.dma_start(out_o[128 * tb:128 * (tb + 1), :], lg_sb[:])

    nc.compile()
    return nc


# ------------------------------------------------------------- host helpers
def _host_tables():
    """cos/sin rope tables in [pair, t] layout, mirroring reference fp32 math."""
    n = np.arange(N, dtype=np.float32)
    q = np.floor(n / 2.0) * 2.0
    freqs = (1.0 / (np.float32(THETA) ** (q / np.float32(N)))
             / np.float32(2.0 * math.pi)).astype(np.float32)
    t = np.arange(T, dtype=np.float32)
    phases = (t[:, None] * freqs[None, :]) % 1.0
    phases = phases * np.float32(2.0 * math.pi)
    cos = np.cos(phases).astype(np.float32)   # [T, N]
    sin = np.sin(phases).astype(np.float32)
    # pair p uses freq of n=2p; table[p, t]
    cos_p = cos[:, 0::2].T.copy()  # [N//2, T]
    sin_p = sin[:, 0::2].T.copy()
    return cos_p, sin_p


def _perm_local():
    """Local latent permutation: position -> (pair index, odd flag)."""
    pos_to_pair = np.empty(NHALF, dtype=np.int64)
    pos_is_odd = np.empty(NHALF, dtype=np.int64)
    for j in range(NJ):
        pr = np.arange(128) + 128 * j
        pos_to_pair[256 * j:256 * j + 128] = pr
        pos_is_odd[256 * j:256 * j + 128] = 0
        pos_to_pair[256 * j + 128:256 * j + 256] = pr
        pos_is_odd[256 * j + 128:256 * j + 256] = 1
    return pos_to_pair, pos_is_odd


_NC_CACHE = {}


def _get_nc():
    if "nc" not in _NC_CACHE:
        _NC_CACHE["nc"] = build_program()
    return _NC_CACHE["nc"]


def prepare_in_maps(idx, embed, encoder, encoder_v, decoder, lm_head):
    idx = np.asarray(idx)
    embed = np.asarray(embed, dtype=np.float32)
    encoder = np.asarray(encoder, dtype=np.float32)
    encoder_v = np.asarray(encoder_v, dtype=np.float32)
    decoder = np.asarray(decoder, dtype=np.float32)
    lm_head = np.asarray(lm_head, dtype=np.float32)

    cos_p, sin_p = _host_tables()
    pos_to_pair, pos_is_odd = _perm_local()

    cmask = (np.arange(128)[:, None] < np.arange(128)[None, :]).astype(np.float16)
    ident = np.eye(128, dtype=np.float16)
    ident32 = np.eye(128, dtype=np.float32)
    idx32 = idx.reshape(T).astype(np.float32).reshape(T, 1)
    lmh16 = lm_head.astype(np.float16)

    in_maps = []
    for c in range(NCORES):
        h, eta = c // 2, c % 2
        pair_g = NPAIR * eta + pos_to_pair          # global pair index
        n_orig = 2 * pair_g + pos_is_odd            # original n within head
        enc_sh = encoder[h][:, n_orig].astype(np.float16)
        encv_sh = encoder_v[h][:, n_orig].astype(np.float16)
        dec_sh = decoder[h * N + n_orig, :].astype(np.float16)
        cos_sh = cos_p[NPAIR * eta:NPAIR * (eta + 1), :].astype(np.float16)
        sin_sh = sin_p[NPAIR * eta:NPAIR * (eta + 1), :].astype(np.float16)
        in_maps.append({
            "idx32": idx32, "embed": embed, "enc_sh": enc_sh,
            "encv_sh": encv_sh, "dec_sh": dec_sh, "lmh": lmh16,
            "cos_sh": cos_sh, "sin_sh": sin_sh, "cmask": cmask,
            "ident": ident, "ident32": ident32,
        })
    return in_maps


def kernel(idx, embed, encoder, encoder_v, decoder, lm_head):
    in_maps = prepare_in_maps(idx, embed, encoder, encoder_v, decoder,
                              lm_head)
    nc = _get_nc()
    res = bass_utils.run_bass_kernel_spmd(nc, in_maps,
                                          core_ids=list(range(NCORES)))
    _NC_CACHE["last_results"] = res
    logits = np.asarray(res.results[0]["logits"], dtype=np.float32)
    return logits.reshape(1, T, VOCAB)



# revision 2
# speedup vs baseline: 1.1265x; 1.1265x over previous
"""Trainium2 Bass kernel for nn_BDH_6313601925221 (sparse_attention).

Model (reference.py):
  x = LN(embed[idx])                                   (B=1, T=1024, D=256)
  repeat 6 layers (shared weights):
    x_sparse = relu(einsum('btd,hdn->bhtn', x, encoder))   N=8192, NH=4
    QR       = rope(x_sparse)                              interleaved-pair rotation
    scores   = einsum('bhtn,bhsn->bhts', QR, QR) * strict_causal
    yKV      = LN(einsum('bhts,bsd->bhtd', scores, x))
    y_sparse = relu(einsum('bhtd,hdn->bhtn', yKV, encoder_v))
    yMLP     = (x_sparse*y_sparse).transpose -> (T, NH*N) @ decoder
    x        = LN(x + LN(yMLP))
  logits = x @ lm_head

Distribution (8 cores): core c = (head h=c//2, latent-half eta=c%2).
Each core computes the encoder/rope/scores path over its 4096 latent dims
(pairwise AllReduce of partial scores within the head pair), duplicates the
small yKV path, then computes y_sparse/xy/decoder over its latent half for
all tokens; the yMLP partials are AllReduced over all 8 ranks in two
t-half chunks so the collective overlaps the remaining compute.

v2 scheduling (vs the v1 baseline):
  - j-loop software-pipelined: scores matmuls for pair j-1 are emitted
    after the encoder matmuls of pair j, so the PE never waits on the
    relu->rope chain and the HAM clock stays warm.
  - all 128x128 fp16 transposes moved from the PE to DMA xbar transposes
    (ykv^T, tail ym^T, x_d16, embedding one-hots).
  - phase 3 (y_sparse/xy/decoder) runs in four 256-column t-chunks:
    chunk A (qb0,1) overlaps the group-1 scores AllReduce; the yMLP
    AllReduce is split per t-half and overlaps chunks C,D / the tail.
  - score strips live in 8 per-kb SBUF tiles; collective bounce
    spill/load is 3 big DMAs per group instead of 36 small ones.
  - decoder weights + lm_head resident in SBUF across all layers.

Layouts: latent dim N is host-permuted so rope pairs are de-interleaved:
local tile 2j = even pair members, 2j+1 = odd. Inner products over N and
the decoder contraction are invariant to this permutation (weights are
permuted to match).

PSUM budget (8 banks): acc_a/acc_b/acc_c [128,1024] f32 (2 banks each,
bufs=1) carry scores strips, then ym accumulators (acc_a/b) and the yKV
chains (acc_c regions); ps_w [128,512]-padded (bufs=2) carries transient
matmul outputs.
"""

import math
import sys

import numpy as np

for _p in ("/opt/trn_rl_repo",):
    if _p not in sys.path:
        sys.path.insert(0, _p)

import concourse.bass as bass
import concourse.mybir as mybir
import concourse.tile as tile
from concourse import bacc
from concourse import bass_utils

# ---------------------------------------------------------------- constants
D = 256
NH = 4
N = 8192
T = 1024
N_LAYER = 6
VOCAB = 256
THETA = 2 ** 16
EPS = 1e-5
NCORES = 8

NHALF = N // 2          # 4096 latent dims per core
NPAIR = NHALF // 2      # 2048 rope pairs per core
NT = NHALF // 128       # 32 local n-tiles of 128
NJ = NT // 2            # 16 pair-blocks (tile 2j = evens, 2j+1 = odds)
TB = T // 128           # 8 token blocks
DC = D // 128           # 2 d-chunks

F16 = mybir.dt.float16
F32 = mybir.dt.float32
I32 = mybir.dt.int32
AX = mybir.AxisListType
ALU = mybir.AluOpType
ACTF = mybir.ActivationFunctionType

# kb -> (group, acc tag, column offset inside the [128,1024] acc tile)
SC_LAYOUT = {
    0: (0, "acc_a", 0),
    1: (0, "acc_b", 0),
    2: (0, "acc_c", 0),
    3: (1, "acc_a", 0),
    4: (1, "acc_b", 0),
    5: (1, "acc_b", 512),
    6: (1, "acc_c", 0),
    7: (1, "acc_c", 512),
}
G0_KBS = [0, 1, 2]
G1_KBS = [3, 4, 5, 6, 7]
# row offset (in 128-row blocks) of strip kb inside its group bounce buffer
G0_ROW = {0: 0, 1: 8, 2: 15}
G0_BLOCKS = 21
G1_ROW = {3: 0, 4: 5, 5: 9, 6: 12, 7: 14}
G1_BLOCKS = 15


def _ln_free(nc, pool, x_ap, eps_ap, out_f32=None, out_f16=None,
             skip_mean=False, n=None, name=""):
    """LayerNorm along the free dim of a [128, n] tile (per-partition stats)."""
    n = n if n is not None else x_ap.shape[-1]
    inv_n = 1.0 / n
    sq = pool.tile([128, n], F32, name=f"lnsq{name}", tag="lnsq")
    ssq = pool.tile([128, 1], F32, name=f"lnssq{name}", tag="lnssq")
    std = pool.tile([128, 1], F32, name=f"lnstd{name}", tag="lnstd")
    inv = pool.tile([128, 1], F32, name=f"lninv{name}", tag="lninv")
    if skip_mean:
        xm = x_ap
    else:
        mu = pool.tile([128, 1], F32, name=f"lnmu{name}", tag="lnmu")
        xm_t = pool.tile([128, n], F32, name=f"lnxm{name}", tag="lnxm")
        nc.vector.tensor_reduce(mu[:], x_ap, axis=AX.X, op=ALU.add)
        nc.scalar.mul(mu[:], mu[:], inv_n)
        nc.vector.tensor_scalar_sub(xm_t[:], x_ap, mu[:])
        xm = xm_t[:]
    nc.scalar.activation(sq[:], xm, ACTF.Square, accum_out=ssq[:])
    nc.scalar.activation(std[:], ssq[:], ACTF.Sqrt, bias=eps_ap, scale=inv_n)
    nc.vector.reciprocal(inv[:], std[:])
    if out_f32 is not None:
        nc.vector.tensor_scalar_mul(out_f32, xm, inv[:])
    if out_f16 is not None:
        nc.scalar.activation(out_f16, xm, ACTF.Copy, scale=inv[:])
    return xm, inv


def build_program(n_layer=N_LAYER):
    nc = bacc.Bacc("TRN2", target_bir_lowering=False, debug=False,
                   num_devices=NCORES)

    # ------------------------------------------------------------- I/O decl
    idx_i = nc.dram_tensor("idx32", [T, 1], F32, kind="ExternalInput")
    embed_i = nc.dram_tensor("embed", [VOCAB, D], F32, kind="ExternalInput")
    enc_i = nc.dram_tensor("enc_sh", [D, NHALF], F16, kind="ExternalInput")
    encv_i = nc.dram_tensor("encv_sh", [D, NHALF], F16, kind="ExternalInput")
    dec_i = nc.dram_tensor("dec_sh", [NHALF, D], F16, kind="ExternalInput")
    lmh_i = nc.dram_tensor("lmh", [D, VOCAB], F16, kind="ExternalInput")
    cos_i = nc.dram_tensor("cos_sh", [NPAIR, T], F16, kind="ExternalInput")
    sin_i = nc.dram_tensor("sin_sh", [NPAIR, T], F16, kind="ExternalInput")
    cmask_i = nc.dram_tensor("cmask", [128, 128], F16, kind="ExternalInput")
    out_o = nc.dram_tensor("logits", [T, VOCAB], F32, kind="ExternalOutput")

    pair_groups = [[2 * h, 2 * h + 1] for h in range(NH)]
    all_group = [list(range(NCORES))]

    with tile.TileContext(nc) as tc:
      with (
        tc.tile_pool(name="persist", bufs=1) as pp,
        tc.tile_pool(name="work", bufs=2) as wp,
        tc.tile_pool(name="psW", bufs=2, space="PSUM") as psW,
        tc.tile_pool(name="psAcc", bufs=1, space="PSUM") as psAcc,
        tc.tile_pool(name="dram", bufs=1, space="DRAM") as dp,
      ):
        # ------------------------------------------------- persistent SBUF
        enc_sb = [pp.tile([128, NHALF], F16, name=f"enc{d}", tag=f"enc{d}")
                  for d in range(DC)]
        encv_sb = [pp.tile([128, NHALF], F16, name=f"encv{d}", tag=f"encv{d}")
                   for d in range(DC)]
        dec_sb = [pp.tile([128, D], F16, name=f"dec{i}", tag=f"dec{i}")
                  for i in range(NT)]
        QR = [pp.tile([128, T], F16, name=f"qr{i}", tag=f"qr{i}")
              for i in range(NT)]
        # score strips S^T[kb]: [128 s, (TB-kb)*128 q] fp16, diag-masked
        ST = [pp.tile([128, (TB - kb) * 128], F16, name=f"st{kb}",
                      tag=f"st{kb}") for kb in range(TB)]
        x_t32 = [pp.tile([128, D], F32, name=f"xt32_{i}", tag=f"xt32_{i}")
                 for i in range(TB)]
        x_t16 = [pp.tile([128, D], F16, name=f"xt16_{i}", tag=f"xt16_{i}")
                 for i in range(TB)]
        x_d16 = [pp.tile([128, T], F16, name=f"xd16_{i}", tag=f"xd16_{i}")
                 for i in range(DC)]
        ykv_t = [pp.tile([128, D], F16, name=f"ykvt{i}", tag=f"ykvt{i}")
                 for i in range(TB)]
        ykvT = [pp.tile([128, T], F16, name=f"ykvT{i}", tag=f"ykvT{i}")
                for i in range(DC)]
        cmask = pp.tile([128, 128], F16, name="cmaskt", tag="cmaskt")
        eps_t = pp.tile([128, 1], F32, name="eps_t", tag="eps_t")
        lmh_sb = [pp.tile([128, VOCAB], F16, name=f"lmh{d}", tag=f"lmh{d}")
                  for d in range(DC)]

        # ---------------------------------------------------- DRAM buffers
        xs_spill = dp.tile([NHALF, T], F16, name="xs_spill")
        sc_in0 = dp.tile([G0_BLOCKS * 128, 128], F16, name="sc_in0")
        sc_out0 = dp.tile([G0_BLOCKS * 128, 128], F16, name="sc_out0")
        sc_in1 = dp.tile([G1_BLOCKS * 128, 128], F16, name="sc_in1")
        sc_out1 = dp.tile([G1_BLOCKS * 128, 128], F16, name="sc_out1")
        ym_ins = [dp.tile([D, 512], F16, name=f"ym_in{h}", tag=f"ym_in{h}")
                  for h in range(2)]
        ym_outs = [[dp.tile([D, 512], F16, name=f"ym_out{l}_{h}",
                            tag=f"ym_out{l}_{h}", addr_space="Shared")
                    for h in range(2)] for l in range(n_layer)]

        def psw(name, shape=(128, 512), dtype=F32):
            return psW.tile(list(shape), dtype, name=name, tag="ps_w",
                            padded_shape=[128, 512])

        # ------------------------------------------------------ load consts
        nc.gpsimd.memset(eps_t[:], EPS)
        nc.sync.dma_start(cmask[:], cmask_i[:, :])
        for d in range(DC):
            nc.sync.dma_start(enc_sb[d][:], enc_i[128 * d:128 * (d + 1), :])
            nc.sync.dma_start(encv_sb[d][:], encv_i[128 * d:128 * (d + 1), :])
            nc.sync.dma_start(lmh_sb[d][:], lmh_i[128 * d:128 * (d + 1), :])
        for i in range(NT):
            nc.scalar.dma_start(dec_sb[i][:], dec_i[128 * i:128 * (i + 1), :])

        # ------------------------------------------------------- embedding
        # E_n = LN(embed) per vocab row; x0 = onehot(idx) @ E_n
        with tc.tile_pool(name="embed", bufs=1) as ep:
            E_n = [ep.tile([128, D], F16, name=f"en{v}", tag=f"en{v}")
                   for v in range(DC)]
            for v in range(DC):
                emb_raw = ep.tile([128, D], F32, name=f"emb_raw{v}",
                                  tag=f"emb_raw{v}")
                nc.sync.dma_start(emb_raw[:], embed_i[128 * v:128 * (v + 1), :])
                _ln_free(nc, wp, emb_raw[:], eps_t[:], out_f16=E_n[v][:],
                         name=f"emb{v}")

            iota_i32 = ep.tile([128, VOCAB], I32, name="iota_i32",
                               tag="iota_i32")
            nc.gpsimd.iota(iota_i32[:], pattern=[[1, VOCAB]], base=0,
                           channel_multiplier=0)
            iota_t = ep.tile([128, VOCAB], F32, name="iota_t", tag="iota_t")
            nc.vector.tensor_copy(iota_t[:], iota_i32[:])
            OHT = [ep.tile([128, T], F16, name=f"oht{v}", tag=f"oht{v}")
                   for v in range(DC)]
            for tb in range(TB):
                idx_col = wp.tile([128, 1], F32, name="idx_col", tag="idx_col")
                nc.sync.dma_start(idx_col[:], idx_i[128 * tb:128 * (tb + 1), :])
                oh_tm = wp.tile([128, VOCAB], F16, name="oh_tm", tag="oh_tm",
                                bufs=3)
                nc.vector.tensor_scalar(oh_tm[:], iota_t[:], idx_col[:], None,
                                        op0=ALU.is_equal)
                for v in range(DC):
                    nc.sync.dma_start_transpose(
                        OHT[v][:, 128 * tb:128 * (tb + 1)],
                        oh_tm[:, 128 * v:128 * (v + 1)])

            for tb in range(TB):
                ps_x = psw(f"ps_x0_{tb}", (128, D))
                for v in range(DC):
                    nc.tensor.matmul(ps_x[:],
                                     OHT[v][:, 128 * tb:128 * (tb + 1)],
                                     E_n[v][:], start=(v == 0),
                                     stop=(v == DC - 1))
                nc.vector.tensor_copy(x_t32[tb][:], ps_x[:])
                nc.scalar.copy(x_t16[tb][:], ps_x[:])
            for d in range(DC):
                for th in range(2):
                    ps_xd = psw(f"ps_xd_{d}_{th}")
                    for v in range(DC):
                        nc.tensor.matmul(
                            ps_xd[:], E_n[v][:, 128 * d:128 * (d + 1)],
                            OHT[v][:, 512 * th:512 * (th + 1)],
                            start=(v == 0), stop=(v == DC - 1))
                    nc.scalar.copy(x_d16[d][:, 512 * th:512 * (th + 1)],
                                   ps_xd[:])

        # ============================================================ layers
        for layer in range(n_layer):
            # ---------------- phase 1a: x_sparse + rope + scores group 0
            acc = {t: psAcc.tile([128, 1024], F32, name=f"{t}_s0_{layer}",
                                 tag=t) for t in ("acc_a", "acc_b", "acc_c")}

            def sc_ap(kb, grp_acc):
                _, tag, off = SC_LAYOUT[kb]
                w = (TB - kb) * 128
                return grp_acc[tag][:, off:off + w]

            def scores_block(nt, kbs, grp_acc):
                for kb in kbs:
                    dst = sc_ap(kb, grp_acc)
                    w = (TB - kb) * 128
                    for nn in range(0, w, 512):
                        nw = min(512, w - nn)
                        nc.tensor.matmul(
                            dst[:, nn:nn + nw],
                            QR[nt][:, 128 * kb:128 * (kb + 1)],
                            QR[nt][:, 128 * kb + nn:128 * kb + nn + nw],
                            start=(nt == 0), stop=(nt == NT - 1))

            for j in range(NJ):
                ct = wp.tile([128, T], F16, name="cos_t", tag="cos_t", bufs=3)
                st_t = wp.tile([128, T], F16, name="sin_t", tag="sin_t",
                               bufs=3)
                nc.sync.dma_start(ct[:], cos_i[128 * j:128 * (j + 1), :])
                nc.sync.dma_start(st_t[:], sin_i[128 * j:128 * (j + 1), :])
                xs_pair = []
                for par in range(2):  # even tile, odd tile
                    nt = 2 * j + par
                    xs_sb = wp.tile([128, T], F16, name="xs_sb", tag="xs_sb",
                                    bufs=4)
                    for th in range(2):
                        ps_e = psw(f"ps_enc_{layer}_{nt}_{th}")
                        for d in range(DC):
                            nc.tensor.matmul(
                                ps_e[:],
                                enc_sb[d][:, 128 * nt:128 * (nt + 1)],
                                x_d16[d][:, 512 * th:512 * (th + 1)],
                                start=(d == 0), stop=(d == DC - 1))
                        nc.scalar.activation(xs_sb[:, 512 * th:512 * (th + 1)],
                                             ps_e[:], ACTF.Relu)
                    nc.sync.dma_start(
                        xs_spill[128 * nt:128 * (nt + 1), :], xs_sb[:])
                    xs_pair.append(xs_sb)
                # rope: qr_e = xs_e*c - xs_o*s ; qr_o = xs_o*c + xs_e*s
                xe, xo = xs_pair[0], xs_pair[1]
                qe, qo = QR[2 * j], QR[2 * j + 1]
                p1 = wp.tile([128, T], F16, name="rp1", tag="rp1")
                p2 = wp.tile([128, T], F16, name="rp2", tag="rp2")
                nc.vector.tensor_mul(p1[:], xe[:], ct[:])
                nc.gpsimd.tensor_mul(p2[:], xo[:], st_t[:])
                nc.vector.tensor_sub(qe[:], p1[:], p2[:])
                nc.vector.tensor_mul(p1[:], xo[:], ct[:])
                nc.gpsimd.tensor_mul(p2[:], xe[:], st_t[:])
                nc.vector.tensor_add(qo[:], p1[:], p2[:])
                # scores for the PREVIOUS pair (software pipeline: PE stays
                # busy on these while this pair's relu/rope chain completes)
                if j > 0:
                    scores_block(2 * (j - 1), G0_KBS, acc)
                    scores_block(2 * (j - 1) + 1, G0_KBS, acc)
            scores_block(2 * (NJ - 1), G0_KBS, acc)
            scores_block(2 * (NJ - 1) + 1, G0_KBS, acc)

            # spill score group 0 to DRAM bounce (fp16, one DMA per strip)
            def spill_group(kbs, grp_acc, row_of, sc_in):
                for kb in kbs:
                    w = (TB - kb) * 128
                    s_sb = wp.tile([128, w], F16, name=f"s_sb{kb}",
                                   tag="s_sb", padded_shape=[128, 1024])
                    nc.scalar.copy(s_sb[:], sc_ap(kb, grp_acc))
                    dst = sc_in[:].rearrange("(b p) n -> p b n", p=128)
                    nc.sync.dma_start(
                        dst[:, row_of[kb]:row_of[kb] + (TB - kb), :],
                        s_sb[:].rearrange("p (b n) -> p b n", n=128))

            spill_group(G0_KBS, acc, G0_ROW, sc_in0)
            # AR of group 0 overlaps with the group-1 matmuls below
            nc.gpsimd.collective_compute(
                "AllReduce", ALU.add, replica_groups=pair_groups,
                ins=[sc_in0.opt()], outs=[sc_out0.opt()])

            # ---------------- phase 1b: scores group 1 (QR resident)
            acc1 = {t: psAcc.tile([128, 1024], F32, name=f"{t}_s1_{layer}",
                                  tag=t) for t in ("acc_a", "acc_b", "acc_c")}
            for nt in range(NT):
                scores_block(nt, G1_KBS, acc1)
            spill_group(G1_KBS, acc1, G1_ROW, sc_in1)
            nc.gpsimd.collective_compute(
                "AllReduce", ALU.add, replica_groups=pair_groups,
                ins=[sc_in1.opt()], outs=[sc_out1.opt()])

            # load group-0 strips (waits on AR0) + mask diag blocks
            def load_group(kbs, row_of, sc_out):
                for kb in kbs:
                    src = sc_out[:].rearrange("(b p) n -> p b n", p=128)
                    nc.sync.dma_start(
                        ST[kb][:].rearrange("p (b n) -> p b n", n=128),
                        src[:, row_of[kb]:row_of[kb] + (TB - kb), :])
                    nc.vector.tensor_mul(ST[kb][:, 0:128], ST[kb][:, 0:128],
                                         cmask[:])

            load_group(G0_KBS, G0_ROW, sc_out0)

            # ---------------- phase 2/3 interleaved
            ykv_region = {}  # qb -> (tile, col)  spread across psum banks
            ykv_tiles = [
                psAcc.tile([128, 1024], F32, name=f"ykv_ps_{layer}_0",
                           tag="acc_c"),
                psAcc.tile([128, 1024], F32, name=f"ykv_ps_{layer}_1",
                           tag="acc_c"),
            ]
            for qb in range(TB):
                col = 512 * (qb % 2) + 256 * ((qb // 2) % 2)
                ykv_region[qb] = (ykv_tiles[qb // 4], col)

            def ykv_qb(qb):
                tl, col = ykv_region[qb]
                ps_y = tl[:, col:col + D]
                for kb in range(qb + 1):
                    nc.tensor.matmul(
                        ps_y, ST[kb][:, 128 * (qb - kb):128 * (qb - kb + 1)],
                        x_t16[kb][:], start=(kb == 0), stop=(kb == qb))
                # x rows are zero-mean (LN output) => yKV rows are zero-mean
                _ln_free(nc, wp, ps_y, eps_t[:], out_f16=ykv_t[qb][:],
                         skip_mean=True, name=f"ykv{qb}")
                for d in range(DC):
                    nc.scalar.dma_start_transpose(
                        ykvT[d][:, 128 * qb:128 * (qb + 1)],
                        ykv_t[qb][:, 128 * d:128 * (d + 1)])

            ym_acc = {}
            for dh in range(DC):
                t = ("acc_a", "acc_b")[dh]
                ym_acc[dh] = psAcc.tile([128, 1024], F32,
                                        name=f"{t}_ym_{layer}", tag=t)

            def phase3_chunk(c):
                """encv+xy+decoder for t columns [256c, 256c+256)."""
                lo = 256 * c
                prev = None
                for nt in range(NT):
                    ps_v = psw(f"ps_ysp_{layer}_{nt}_{c}", (128, 256))
                    for d in range(DC):
                        nc.tensor.matmul(
                            ps_v[:], encv_sb[d][:, 128 * nt:128 * (nt + 1)],
                            ykvT[d][:, lo:lo + 256],
                            start=(d == 0), stop=(d == DC - 1))
                    xs_c = wp.tile([128, 256], F16, name="xs_c", tag="xs_c",
                                   bufs=4)
                    nc.scalar.dma_start(
                        xs_c[:],
                        xs_spill[128 * nt:128 * (nt + 1), lo:lo + 256])
                    xy = wp.tile([128, 256], F16, name="xy", tag="xy", bufs=3)
                    # xy = relu(ys) * xs  (fused)
                    nc.vector.scalar_tensor_tensor(
                        xy[:], ps_v[:], 0.0, xs_c[:],
                        op0=ALU.max, op1=ALU.mult)
                    if prev is not None:
                        emit_ym(*prev)
                    prev = (nt, c, xy)
                emit_ym(*prev)

            def emit_ym(nt, c, xy):
                lo = 256 * c
                for dh in range(DC):
                    nc.tensor.matmul(
                        ym_acc[dh][:, lo:lo + 256],
                        dec_sb[nt][:, 128 * dh:128 * (dh + 1)],
                        xy[:], start=(nt == 0), stop=(nt == NT - 1))

            # chunk A (qb0,1) overlaps the group-1 AllReduce
            for qb in range(3):
                ykv_qb(qb)
            phase3_chunk(0)

            # group-1 strips land; remaining yKV rows
            load_group(G1_KBS, G1_ROW, sc_out1)
            for qb in range(3, TB):
                ykv_qb(qb)
            phase3_chunk(1)

            # ---------------- yMLP AllReduce, t-half 0
            for dh in range(DC):
                ym_sb = wp.tile([128, 512], F16, name="ym_sb0", tag="ym_sb",
                                bufs=2)
                nc.scalar.copy(ym_sb[:], ym_acc[dh][:, 0:512])
                nc.sync.dma_start(ym_ins[0][128 * dh:128 * (dh + 1), :],
                                  ym_sb[:])
            nc.gpsimd.collective_compute(
                "AllReduce", ALU.add, replica_groups=all_group,
                ins=[ym_ins[0].opt()], outs=[ym_outs[layer][0].opt()])

            phase3_chunk(2)
            phase3_chunk(3)

            # ---------------- yMLP AllReduce, t-half 1
            for dh in range(DC):
                ym_sb = wp.tile([128, 512], F16, name="ym_sb1", tag="ym_sb",
                                bufs=2)
                nc.scalar.copy(ym_sb[:], ym_acc[dh][:, 512:1024])
                nc.sync.dma_start(ym_ins[1][128 * dh:128 * (dh + 1), :],
                                  ym_sb[:])
            nc.gpsimd.collective_compute(
                "AllReduce", ALU.add, replica_groups=all_group,
                ins=[ym_ins[1].opt()], outs=[ym_outs[layer][1].opt()])

            # ---------------- tail: x = LN(x + LN(yMLP)), per t-half
            for half in range(2):
                ym_out = ym_outs[layer][half]
                for tb in range(4 * half, 4 * half + 4):
                    u = wp.tile([128, D], F16, name="u_t", tag="u_t", bufs=3)
                    nc.sync.dma_start_transpose(
                        u[:], ym_out[:, 128 * (tb - 4 * half):
                                     128 * (tb - 4 * half) + 128])
                    xm_u, inv_u = _ln_free(nc, wp, u[:], eps_t[:],
                                           name=f"u{tb}")
                    v = wp.tile([128, D], F32, name="v_t", tag="v_t")
                    nc.vector.scalar_tensor_tensor(
                        v[:], xm_u, inv_u[:], x_t32[tb][:],
                        op0=ALU.mult, op1=ALU.add)
                    _ln_free(nc, wp, v[:], eps_t[:], out_f32=x_t32[tb][:],
                             out_f16=x_t16[tb][:], skip_mean=True,
                             name=f"v{tb}")
                    for d in range(DC):
                        nc.sync.dma_start_transpose(
                            x_d16[d][:, 128 * tb:128 * (tb + 1)],
                            x_t16[tb][:, 128 * d:128 * (d + 1)])

        # ------------------------------------------------------- lm head
        for tb in range(TB):
            ps_l = psw(f"ps_lg_{tb}", (128, VOCAB))
            for d in range(DC):
                nc.tensor.matmul(ps_l[:], x_d16[d][:, 128 * tb:128 * (tb + 1)],
                                 lmh_sb[d][:], start=(d == 0),
                                 stop=(d == DC - 1))
            lg_sb = wp.tile([128, VOCAB], F32, name="lg_sb", tag="lg_sb")
            nc.vector.tensor_copy(lg_sb[:], ps_l[:])
            nc.sync.dma_start(out_o[128 * tb:128 * (tb + 1), :], lg_sb[:])

    nc.compile()
    return nc


# ------------------------------------------------------------- host helpers
def _host_tables():
    """cos/sin rope tables in [pair, t] layout, mirroring reference fp32 math."""
    n = np.arange(N, dtype=np.float32)
    q = np.floor(n / 2.0) * 2.0
    freqs = (1.0 / (np.float32(THETA) ** (q / np.float32(N)))
             / np.float32(2.0 * math.pi)).astype(np.float32)
    t = np.arange(T, dtype=np.float32)
    phases = (t[:, None] * freqs[None, :]) % 1.0
    phases = phases * np.float32(2.0 * math.pi)
    cos = np.cos(phases).astype(np.float32)   # [T, N]
    sin = np.sin(phases).astype(np.float32)
    # pair p uses freq of n=2p; table[p, t]
    cos_p = cos[:, 0::2].T.copy()  # [N//2, T]
    sin_p = sin[:, 0::2].T.copy()
    return cos_p, sin_p


def _perm_local():
    """Local latent permutation: position -> (pair index, odd flag)."""
    pos_to_pair = np.empty(NHALF, dtype=np.int64)
    pos_is_odd = np.empty(NHALF, dtype=np.int64)
    for j in range(NJ):
        pr = np.arange(128) + 128 * j
        pos_to_pair[256 * j:256 * j + 128] = pr
        pos_is_odd[256 * j:256 * j + 128] = 0
        pos_to_pair[256 * j + 128:256 * j + 256] = pr
        pos_is_odd[256 * j + 128:256 * j + 256] = 1
    return pos_to_pair, pos_is_odd


_NC_CACHE = {}


def _get_nc():
    if "nc" not in _NC_CACHE:
        _NC_CACHE["nc"] = build_program()
    return _NC_CACHE["nc"]


def prepare_in_maps(idx, embed, encoder, encoder_v, decoder, lm_head):
    idx = np.asarray(idx)
    embed = np.asarray(embed, dtype=np.float32)
    encoder = np.asarray(encoder, dtype=np.float32)
    encoder_v = np.asarray(encoder_v, dtype=np.float32)
    decoder = np.asarray(decoder, dtype=np.float32)
    lm_head = np.asarray(lm_head, dtype=np.float32)

    cos_p, sin_p = _host_tables()
    pos_to_pair, pos_is_odd = _perm_local()

    cmask = (np.arange(128)[:, None] < np.arange(128)[None, :]).astype(np.float16)
    idx32 = idx.reshape(T).astype(np.float32).reshape(T, 1)
    lmh16 = lm_head.astype(np.float16)

    in_maps = []
    for c in range(NCORES):
        h, eta = c // 2, c % 2
        pair_g = NPAIR * eta + pos_to_pair          # global pair index
        n_orig = 2 * pair_g + pos_is_odd            # original n within head
        enc_sh = encoder[h][:, n_orig].astype(np.float16)
        encv_sh = encoder_v[h][:, n_orig].astype(np.float16)
        dec_sh = decoder[h * N + n_orig, :].astype(np.float16)
        cos_sh = cos_p[NPAIR * eta:NPAIR * (eta + 1), :].astype(np.float16)
        sin_sh = sin_p[NPAIR * eta:NPAIR * (eta + 1), :].astype(np.float16)
        in_maps.append({
            "idx32": idx32, "embed": embed, "enc_sh": enc_sh,
            "encv_sh": encv_sh, "dec_sh": dec_sh, "lmh": lmh16,
            "cos_sh": cos_sh, "sin_sh": sin_sh, "cmask": cmask,
        })
    return in_maps


def kernel(idx, embed, encoder, encoder_v, decoder, lm_head):
    in_maps = prepare_in_maps(idx, embed, encoder, encoder_v, decoder,
                              lm_head)
    nc = _get_nc()
    res = bass_utils.run_bass_kernel_spmd(nc, in_maps,
                                          core_ids=list(range(NCORES)))
    _NC_CACHE["last_results"] = res
    logits = np.asarray(res.results[0]["logits"], dtype=np.float32)
    return logits.reshape(1, T, VOCAB)


# revision 16
# speedup vs baseline: 1.5060x; 1.3369x over previous
"""Trainium2 Bass kernel for nn_BDH_6313601925221 (sparse_attention).

Model (reference.py):
  x = LN(embed[idx])                                   (B=1, T=1024, D=256)
  repeat 6 layers (shared weights):
    x_sparse = relu(einsum('btd,hdn->bhtn', x, encoder))   N=8192, NH=4
    QR       = rope(x_sparse)                              interleaved-pair rotation
    scores   = einsum('bhtn,bhsn->bhts', QR, QR) * strict_causal
    yKV      = LN(einsum('bhts,bsd->bhtd', scores, x))
    y_sparse = relu(einsum('bhtd,hdn->bhtn', yKV, encoder_v))
    yMLP     = (x_sparse*y_sparse).transpose -> (T, NH*N) @ decoder
    x        = LN(x + LN(yMLP))
  logits = x @ lm_head

Distribution (8 cores): core c = (head h=c//2, latent-half eta=c%2).
Each core computes the encoder/rope/scores path over its 4096 latent dims
(pairwise AllReduce of partial scores within the head pair), duplicates the
small yKV path, then computes y_sparse/xy/decoder over its latent half for
all tokens; the yMLP partials are AllReduced over all 8 ranks in two
t-half chunks so the collective overlaps the remaining compute.

Key algebraic simplification: the inner LN on yKV can be dropped entirely.
LN is scale-invariant per row, relu is positively homogeneous, and the
whole yKV->y_sparse->xy->decoder path is linear in the per-token scale, so
the per-token 1/std (and the exactly-zero mean, since x rows are zero-mean
LN outputs) cancels inside the tail LN(yMLP). This lets yKV be produced
DIRECTLY in transposed [d, t] layout by the PE (lhsT = x tiles, rhs = score
strips) with no LayerNorm, no transposes.

Scheduling:
  - j-loop software-pipelined: scores matmuls for pair j-1 are emitted
    after the encoder matmuls of pair j, so the PE never waits on the
    relu->rope chain and the HAM clock stays warm.
  - rope runs on wide [128,2048] paired tiles (xs2=[xe|xo] against
    cs2=[c|s], sc2=[s|c]): 3 DVE ops + 1 GpSimd op per pair.
  - phase 3 (y_sparse/xy/decoder) runs in two 512-wide t-half passes;
    the yMLP AllReduce is split per t-half and overlaps the other pass /
    the tail. Score-strip collective bounce is 3 big DMAs per group.
  - decoder weights + lm_head resident in SBUF across all layers.

PSUM budget (8 banks): acc_a/acc_b/acc_c [128,1024] f32 (2 banks each,
bufs=1) carry scores strips, then ym accumulators (acc_a/b) and the yKV^T
accumulators (acc_c); ps_w [128,512]-padded (bufs=2) carries transient
matmul outputs.
"""

import math
import sys

import numpy as np

for _p in ("/opt/trn_rl_repo",):
    if _p not in sys.path:
        sys.path.insert(0, _p)

import concourse.bass as bass
import concourse.mybir as mybir
import concourse.tile as tile
from concourse import bacc
from concourse import bass_utils

# ---------------------------------------------------------------- constants
D = 256
NH = 4
N = 8192
T = 1024
N_LAYER = 6
VOCAB = 256
THETA = 2 ** 16
EPS = 1e-5
NCORES = 8

NHALF = N // 2          # 4096 latent dims per core
NPAIR = NHALF // 2      # 2048 rope pairs per core
NT = NHALF // 128       # 32 local n-tiles of 128
NJ = NT // 2            # 16 pair-blocks (tile 2j = evens, 2j+1 = odds)
TB = T // 128           # 8 token blocks
DC = D // 128           # 2 d-chunks

F16 = mybir.dt.float16
F32 = mybir.dt.float32
I32 = mybir.dt.int32
AX = mybir.AxisListType
ALU = mybir.AluOpType
ACTF = mybir.ActivationFunctionType

# kb -> (group, acc tag, column offset inside the [128,1024] acc tile)
SC_LAYOUT = {
    0: (0, "acc_a", 0),
    1: (0, "acc_b", 0),
    2: (0, "acc_c", 0),
    3: (1, "acc_a", 0),
    4: (1, "acc_b", 0),
    5: (1, "acc_b", 512),
    6: (1, "acc_c", 0),
    7: (1, "acc_c", 512),
}
ALPHA = 1.0 / 512.0     # yKV pre-scale (overflow headroom; cancels exactly)
BETA = 1.0 / 16.0       # extra scale inside Square so sq fits fp16
G0_KBS = [0, 1, 2]
G1_KBS = [3, 4, 5, 6, 7]
# row offset (in 128-row blocks) of strip kb inside its group bounce buffer
G0_ROW = {0: 0, 1: 8, 2: 15}
G0_BLOCKS = 21
G1_ROW = {3: 0, 4: 5, 5: 9, 6: 12, 7: 14}
G1_BLOCKS = 15


def _ln_free(nc, pool, x_ap, eps_ap, out_f32=None, out_f16=None,
             skip_mean=False, n=None, name=""):
    """LayerNorm along the free dim of a [128, n] tile (per-partition stats)."""
    n = n if n is not None else x_ap.shape[-1]
    inv_n = 1.0 / n
    sq = pool.tile([128, n], F16, name=f"lnsq{name}", tag="lnsq")
    ssq = pool.tile([128, 1], F32, name=f"lnssq{name}", tag="lnssq")
    std = pool.tile([128, 1], F32, name=f"lnstd{name}", tag="lnstd")
    inv = pool.tile([128, 1], F32, name=f"lninv{name}", tag="lninv")
    if skip_mean:
        xm = x_ap
    else:
        mu = pool.tile([128, 1], F32, name=f"lnmu{name}", tag="lnmu")
        xm_t = pool.tile([128, n], F32, name=f"lnxm{name}", tag="lnxm")
        nc.vector.tensor_reduce(mu[:], x_ap, axis=AX.X, op=ALU.add)
        nc.scalar.mul(mu[:], mu[:], inv_n)
        nc.vector.tensor_scalar_sub(xm_t[:], x_ap, mu[:])
        xm = xm_t[:]
    nc.scalar.activation(sq[:], xm, ACTF.Square, accum_out=ssq[:])
    nc.scalar.activation(std[:], ssq[:], ACTF.Sqrt, bias=eps_ap, scale=inv_n)
    nc.vector.reciprocal(inv[:], std[:])
    if out_f32 is not None:
        nc.vector.tensor_scalar_mul(out_f32, xm, inv[:])
    if out_f16 is not None:
        nc.scalar.activation(out_f16, xm, ACTF.Copy, scale=inv[:])
    return xm, inv


def build_program(n_layer=N_LAYER):
    nc = bacc.Bacc("TRN2", target_bir_lowering=False, debug=False,
                   num_devices=NCORES)

    # ------------------------------------------------------------- I/O decl
    idx_i = nc.dram_tensor("idx32", [T, 1], F32, kind="ExternalInput")
    embed_i = nc.dram_tensor("embed", [VOCAB, D], F32, kind="ExternalInput")
    enc_i = nc.dram_tensor("enc_sh", [D, NHALF], F16, kind="ExternalInput")
    encv_i = nc.dram_tensor("encv_sh", [D, NHALF], F16, kind="ExternalInput")
    dec_i = nc.dram_tensor("dec_sh", [NHALF, D], F16, kind="ExternalInput")
    lmh_i = nc.dram_tensor("lmh", [D, VOCAB], F16, kind="ExternalInput")
    cos2_i = nc.dram_tensor("cos2_sh", [NPAIR, 2 * T], F16,
                            kind="ExternalInput")
    cmask_i = nc.dram_tensor("cmask", [128, 128], F16, kind="ExternalInput")
    out_o = nc.dram_tensor("logits", [T, VOCAB], F32, kind="ExternalOutput")

    pair_groups = [[2 * h, 2 * h + 1] for h in range(NH)]
    all_group = [list(range(NCORES))]

    with tile.TileContext(nc) as tc:
      with (
        tc.tile_pool(name="persist", bufs=1) as pp,
        tc.tile_pool(name="work", bufs=2) as wp,
        tc.tile_pool(name="psW", bufs=2, space="PSUM") as psW,
        tc.tile_pool(name="psAcc", bufs=1, space="PSUM") as psAcc,
        tc.tile_pool(name="dram", bufs=1, space="DRAM") as dp,
      ):
        # ------------------------------------------------- persistent SBUF
        enc_sb = [pp.tile([128, NHALF], F16, name=f"enc{d}", tag=f"enc{d}")
                  for d in range(DC)]
        encv_sb = [pp.tile([128, NHALF], F16, name=f"encv{d}", tag=f"encv{d}")
                   for d in range(DC)]
        dec_sb = [pp.tile([128, D], F16, name=f"dec{i}", tag=f"dec{i}")
                  for i in range(NT)]
        QR = [pp.tile([128, T], F16, name=f"qr{i}", tag=f"qr{i}")
              for i in range(NT)]
        # score strips S^T[kb]: [128 s, (TB-kb)*128 q] fp16, diag-masked
        ST = [pp.tile([128, (TB - kb) * 128], F16, name=f"st{kb}",
                      tag=f"st{kb}") for kb in range(TB)]
        x_t16 = [pp.tile([128, D], F16, name=f"xt16_{i}", tag=f"xt16_{i}")
                 for i in range(TB)]
        x_d16 = [pp.tile([128, T], F16, name=f"xd16_{i}", tag=f"xd16_{i}")
                 for i in range(DC)]
        ykvT = [pp.tile([128, T], F16, name=f"ykvT{i}", tag=f"ykvT{i}")
                for i in range(DC)]
        cmask = pp.tile([128, 128], F16, name="cmaskt", tag="cmaskt")
        eps_t = pp.tile([128, 1], F32, name="eps_t", tag="eps_t")
        ones_t = pp.tile([128, 1], F16, name="ones_t", tag="ones_t")
        eps2_t = pp.tile([1, 1], F32, name="eps2_t", tag="eps2_t")
        lmh_sb = [pp.tile([128, VOCAB], F16, name=f"lmh{d}", tag=f"lmh{d}")
                  for d in range(DC)]

        # ---------------------------------------------------- DRAM buffers
        xs_spill = dp.tile([NHALF, T], F16, name="xs_spill")
        sc_in0 = dp.tile([G0_BLOCKS * 128, 128], F16, name="sc_in0")
        sc_out0 = dp.tile([G0_BLOCKS * 128, 128], F16, name="sc_out0")
        sc_in1 = dp.tile([G1_BLOCKS * 128, 128], F16, name="sc_in1")
        sc_out1 = dp.tile([G1_BLOCKS * 128, 128], F16, name="sc_out1")
        ym_ins = [dp.tile([D, 512], F16, name=f"ym_in{h}", tag=f"ym_in{h}")
                  for h in range(2)]
        ym_outs = [[dp.tile([D, 512], F16, name=f"ym_out{l}_{h}",
                            tag=f"ym_out{l}_{h}", addr_space="Shared")
                    for h in range(2)] for l in range(n_layer)]

        def psw(name, shape=(128, 512), dtype=F32):
            return psW.tile(list(shape), dtype, name=name, tag="ps_w",
                            padded_shape=[128, 512])

        # ------------------------------------------------------ load consts
        nc.gpsimd.memset(eps_t[:], EPS)
        nc.gpsimd.memset(ones_t[:], 1.0)
        nc.gpsimd.memset(eps2_t[:], EPS * ALPHA * ALPHA)
        nc.sync.dma_start(cmask[:], cmask_i[:, :])
        for d in range(DC):
            nc.sync.dma_start(enc_sb[d][:], enc_i[128 * d:128 * (d + 1), :])
            nc.sync.dma_start(encv_sb[d][:], encv_i[128 * d:128 * (d + 1), :])
            nc.sync.dma_start(lmh_sb[d][:], lmh_i[128 * d:128 * (d + 1), :])
        for i in range(NT):
            nc.scalar.dma_start(dec_sb[i][:], dec_i[128 * i:128 * (i + 1), :])

        # ------------------------------------------------------- embedding
        # E_n = LN(embed) per vocab row; x0 = onehot(idx) @ E_n
        with tc.tile_pool(name="embed", bufs=1) as ep:
            E_n = [ep.tile([128, D], F16, name=f"en{v}", tag=f"en{v}")
                   for v in range(DC)]
            for v in range(DC):
                emb_raw = ep.tile([128, D], F32, name=f"emb_raw{v}",
                                  tag=f"emb_raw{v}")
                nc.sync.dma_start(emb_raw[:], embed_i[128 * v:128 * (v + 1), :])
                _ln_free(nc, wp, emb_raw[:], eps_t[:], out_f16=E_n[v][:],
                         name=f"emb{v}")

            iota_i32 = ep.tile([128, VOCAB], I32, name="iota_i32",
                               tag="iota_i32")
            nc.gpsimd.iota(iota_i32[:], pattern=[[1, VOCAB]], base=0,
                           channel_multiplier=0)
            iota_t = ep.tile([128, VOCAB], F32, name="iota_t", tag="iota_t")
            nc.vector.tensor_copy(iota_t[:], iota_i32[:])
            OHT = [ep.tile([128, T], F16, name=f"oht{v}", tag=f"oht{v}")
                   for v in range(DC)]
            for tb in range(TB):
                idx_col = wp.tile([128, 1], F32, name="idx_col", tag="idx_col")
                nc.sync.dma_start(idx_col[:], idx_i[128 * tb:128 * (tb + 1), :])
                oh_tm = wp.tile([128, VOCAB], F16, name="oh_tm", tag="oh_tm",
                                bufs=3)
                nc.vector.tensor_scalar(oh_tm[:], iota_t[:], idx_col[:], None,
                                        op0=ALU.is_equal)
                for v in range(DC):
                    nc.sync.dma_start_transpose(
                        OHT[v][:, 128 * tb:128 * (tb + 1)],
                        oh_tm[:, 128 * v:128 * (v + 1)])

            for tb in range(TB):
                ps_x = psw(f"ps_x0_{tb}", (128, D))
                for v in range(DC):
                    nc.tensor.matmul(ps_x[:],
                                     OHT[v][:, 128 * tb:128 * (tb + 1)],
                                     E_n[v][:], start=(v == 0),
                                     stop=(v == DC - 1))
                nc.scalar.copy(x_t16[tb][:], ps_x[:])
            for d in range(DC):
                for th in range(2):
                    ps_xd = psw(f"ps_xd_{d}_{th}")
                    for v in range(DC):
                        nc.tensor.matmul(
                            ps_xd[:], E_n[v][:, 128 * d:128 * (d + 1)],
                            OHT[v][:, 512 * th:512 * (th + 1)],
                            start=(v == 0), stop=(v == DC - 1))
                    nc.scalar.copy(x_d16[d][:, 512 * th:512 * (th + 1)],
                                   ps_xd[:])

        # ============================================================ layers
        for layer in range(n_layer):
            # ---------------- phase 1a: x_sparse + rope + scores group 0
            acc = {t: psAcc.tile([128, 1024], F32, name=f"{t}_s0_{layer}",
                                 tag=t) for t in ("acc_a", "acc_b", "acc_c")}

            def sc_ap(kb, grp_acc):
                _, tag, off = SC_LAYOUT[kb]
                w = (TB - kb) * 128
                return grp_acc[tag][:, off:off + w]

            def scores_block(nt, kbs, grp_acc):
                for kb in kbs:
                    dst = sc_ap(kb, grp_acc)
                    w = (TB - kb) * 128
                    for nn in range(0, w, 512):
                        nw = min(512, w - nn)
                        nc.tensor.matmul(
                            dst[:, nn:nn + nw],
                            QR[nt][:, 128 * kb:128 * (kb + 1)],
                            QR[nt][:, 128 * kb + nn:128 * kb + nn + nw],
                            start=(nt == 0), stop=(nt == NT - 1))

            for j in range(NJ):
                cs2 = wp.tile([128, 2 * T], F16, name="cs2", tag="cs2",
                              bufs=2)
                nc.sync.dma_start(cs2[:], cos2_i[128 * j:128 * (j + 1), :])
                # xs2 = [xs_even | xs_odd] for this pair
                xs2 = wp.tile([128, 2 * T], F16, name="xs2", tag="xs2",
                              bufs=3)
                for par in range(2):
                    nt = 2 * j + par
                    for th in range(2):
                        ps_e = psw(f"ps_enc_{layer}_{nt}_{th}")
                        for d in range(DC):
                            nc.tensor.matmul(
                                ps_e[:],
                                enc_sb[d][:, 128 * nt:128 * (nt + 1)],
                                x_d16[d][:, 512 * th:512 * (th + 1)],
                                start=(d == 0), stop=(d == DC - 1))
                        nc.scalar.activation(
                            xs2[:, 1024 * par + 512 * th:
                                1024 * par + 512 * (th + 1)],
                            ps_e[:], ACTF.Relu)
                nc.sync.dma_start(
                    xs_spill[256 * j:256 * (j + 1), :].rearrange(
                        "(b p) n -> p b n", p=128),
                    xs2[:].rearrange("p (b n) -> p b n", n=T))
                # rope: cs2 = [c|s], xs2 = [xe|xo]
                #   m1 = xs2*cs2 = [xe*c | xo*s];  qe = m1_lo - m1_hi
                #   qo = xo*c + xe*s
                qe, qo = QR[2 * j], QR[2 * j + 1]
                m1 = wp.tile([128, 2 * T], F16, name="m1", tag="rope_m",
                             bufs=2)
                nc.vector.tensor_mul(m1[:], xs2[:], cs2[:])
                nc.vector.tensor_sub(qe[:], m1[:, 0:T], m1[:, T:2 * T])
                m2 = wp.tile([128, 2 * T], F16, name="m2", tag="rope_m",
                             bufs=2)
                nc.vector.tensor_mul(m2[:, 0:T], xs2[:, T:2 * T],
                                     cs2[:, 0:T])
                nc.vector.tensor_mul(m2[:, T:2 * T], xs2[:, 0:T],
                                     cs2[:, T:2 * T])
                nc.gpsimd.tensor_add(qo[:], m2[:, 0:T], m2[:, T:2 * T])
                # scores for the PREVIOUS pair (software pipeline: PE stays
                # busy on these while this pair's relu/rope chain completes)
                if j > 0:
                    scores_block(2 * (j - 1), G0_KBS, acc)
                    scores_block(2 * (j - 1) + 1, G0_KBS, acc)
            scores_block(2 * (NJ - 1), G0_KBS, acc)
            scores_block(2 * (NJ - 1) + 1, G0_KBS, acc)

            # spill score group 0 to DRAM bounce (fp16, one DMA per strip)
            def spill_group(kbs, grp_acc, row_of, sc_in):
                for kb in kbs:
                    w = (TB - kb) * 128
                    s_sb = wp.tile([128, w], F16, name=f"s_sb{kb}",
                                   tag="s_sb", padded_shape=[128, 1024])
                    nc.scalar.copy(s_sb[:], sc_ap(kb, grp_acc))
                    dst = sc_in[:].rearrange("(b p) n -> p b n", p=128)
                    nc.sync.dma_start(
                        dst[:, row_of[kb]:row_of[kb] + (TB - kb), :],
                        s_sb[:].rearrange("p (b n) -> p b n", n=128))

            spill_group(G0_KBS, acc, G0_ROW, sc_in0)
            # AR of group 0 overlaps with the group-1 matmuls below
            nc.gpsimd.collective_compute(
                "AllReduce", ALU.add, replica_groups=pair_groups,
                ins=[sc_in0.opt()], outs=[sc_out0.opt()])

            # ---------------- phase 1b: scores group 1 (QR resident)
            acc1 = {t: psAcc.tile([128, 1024], F32, name=f"{t}_s1_{layer}",
                                  tag=t) for t in ("acc_a", "acc_b", "acc_c")}
            for nt in range(NT):
                scores_block(nt, G1_KBS, acc1)
            spill_group(G1_KBS, acc1, G1_ROW, sc_in1)
            nc.gpsimd.collective_compute(
                "AllReduce", ALU.add, replica_groups=pair_groups,
                ins=[sc_in1.opt()], outs=[sc_out1.opt()])

            # load strips (waits on the ARs) + mask diag blocks
            def load_group(kbs, row_of, sc_out):
                for kb in kbs:
                    src = sc_out[:].rearrange("(b p) n -> p b n", p=128)
                    nc.sync.dma_start(
                        ST[kb][:].rearrange("p (b n) -> p b n", n=128),
                        src[:, row_of[kb]:row_of[kb] + (TB - kb), :])
                    nc.vector.tensor_mul(ST[kb][:, 0:128], ST[kb][:, 0:128],
                                         cmask[:])

            load_group(G0_KBS, G0_ROW, sc_out0)
            load_group(G1_KBS, G1_ROW, sc_out1)

            # ---------------- phase 2: yKV^T directly via PE
            # ykvT[d, q] = sum_s x[s, d] * ST[s, q]; strip kb covers
            # q-columns [128*kb, 1024) of the accumulation.
            for dc in range(DC):
                yps = psAcc.tile([128, 1024], F32, name=f"ykv_ps_{layer}_{dc}",
                                 tag="acc_c")
                for kb in range(TB):
                    lo = 128 * kb
                    # segments aligned to the 512-col PSUM bank boundary
                    segs = []
                    if lo < 512:
                        segs.append((lo, 512))
                        segs.append((512, 1024))
                    else:
                        segs.append((lo, 1024))
                    for a, b in segs:
                        nc.tensor.matmul(
                            yps[:, a:b],
                            x_t16[kb][:, 128 * dc:128 * (dc + 1)],
                            ST[kb][:, a - lo:b - lo],
                            start=(kb == 0), stop=(kb == TB - 1))
                nc.scalar.mul(ykvT[dc][:], yps[:], ALPHA)

            # per-(head,token) 1/std of yKV (the deferred inner LayerNorm;
            # rows are zero-mean so var = E[y^2]). Applied to the ym
            # partials below -- exact, incl. eps (scaled by ALPHA^2).
            ssq_ps = [psW.tile([1, 512], F32, name=f"ssq_{layer}_{th}",
                               tag="ps_w", padded_shape=[128, 512])
                      for th in range(2)]
            for dc in range(DC):
                sqt = wp.tile([128, T], F16, name="sqt", tag="sqt", bufs=1)
                nc.scalar.activation(sqt[:], ykvT[dc][:], ACTF.Square,
                                     scale=BETA)
                for th in range(2):
                    nc.tensor.matmul(ssq_ps[th][:], ones_t[:],
                                     sqt[:, 512 * th:512 * (th + 1)],
                                     start=(dc == 0), stop=(dc == DC - 1))
            inv_b = []
            for th in range(2):
                std_row = wp.tile([1, 512], F32, name="std_row",
                                  tag="std_row", bufs=2)
                nc.scalar.activation(std_row[:], ssq_ps[th][:], ACTF.Sqrt,
                                     bias=eps2_t[:],
                                     scale=1.0 / (D * BETA * BETA))
                inv_row = wp.tile([1, 512], F32, name="inv_row",
                                  tag="inv_row", bufs=2)
                nc.vector.reciprocal(inv_row[:], std_row[:])
                ib = wp.tile([128, 512], F32, name=f"inv_b{th}", tag="inv_b",
                             bufs=2)
                nc.gpsimd.partition_broadcast(ib[:], inv_row[:])
                inv_b.append(ib)

            # ---------------- phase 3: y_sparse, xy, decoder partials
            # two 512-wide t-half passes; yMLP^T partials accumulate in
            # ym_acc[dh][:, 512*th:...]
            ym_acc = {}
            for dh in range(DC):
                t = ("acc_a", "acc_b")[dh]
                ym_acc[dh] = psAcc.tile([128, 1024], F32,
                                        name=f"{t}_ym_{layer}", tag=t)

            def emit_ym(nt, th, xy):
                for dh in range(DC):
                    nc.tensor.matmul(
                        ym_acc[dh][:, 512 * th:512 * (th + 1)],
                        dec_sb[nt][:, 128 * dh:128 * (dh + 1)],
                        xy[:], start=(nt == 0), stop=(nt == NT - 1))

            def phase3_pass(th, dma_eng):
                prev = None
                for nt in range(NT):
                    ps_v = psw(f"ps_ysp_{layer}_{nt}_{th}")
                    for d in range(DC):
                        nc.tensor.matmul(
                            ps_v[:], encv_sb[d][:, 128 * nt:128 * (nt + 1)],
                            ykvT[d][:, 512 * th:512 * (th + 1)],
                            start=(d == 0), stop=(d == DC - 1))
                    xs_c = wp.tile([128, 512], F16, name="xs_c", tag="xs_c",
                                   bufs=3)
                    dma_eng.dma_start(
                        xs_c[:],
                        xs_spill[128 * nt:128 * (nt + 1),
                                 512 * th:512 * (th + 1)])
                    xy = wp.tile([128, 512], F16, name="xy", tag="xy", bufs=3)
                    # xy = relu(ys) * xs  (fused)
                    nc.vector.scalar_tensor_tensor(
                        xy[:], ps_v[:], 0.0, xs_c[:],
                        op0=ALU.max, op1=ALU.mult)
                    if prev is not None:
                        emit_ym(*prev)
                    prev = (nt, th, xy)
                emit_ym(*prev)

            phase3_pass(0, nc.scalar)
            # yMLP AllReduce for t-half 0 overlaps the second pass
            for dh in range(DC):
                ym_sb = wp.tile([128, 512], F16, name="ym_sb0", tag="ym_sb",
                                bufs=2)
                nc.vector.tensor_mul(ym_sb[:], ym_acc[dh][:, 0:512],
                                     inv_b[0][:])
                nc.sync.dma_start(ym_ins[0][128 * dh:128 * (dh + 1), :],
                                  ym_sb[:])
            nc.gpsimd.collective_compute(
                "AllReduce", ALU.add, replica_groups=all_group,
                ins=[ym_ins[0].opt()], outs=[ym_outs[layer][0].opt()])

            phase3_pass(1, nc.scalar)
            for dh in range(DC):
                ym_sb = wp.tile([128, 512], F16, name="ym_sb1", tag="ym_sb",
                                bufs=2)
                nc.vector.tensor_mul(ym_sb[:], ym_acc[dh][:, 512:1024],
                                     inv_b[1][:])
                nc.sync.dma_start(ym_ins[1][128 * dh:128 * (dh + 1), :],
                                  ym_sb[:])
            nc.gpsimd.collective_compute(
                "AllReduce", ALU.add, replica_groups=all_group,
                ins=[ym_ins[1].opt()], outs=[ym_outs[layer][1].opt()])

            # ---------------- tail: x = LN(x + LN(yMLP)), per t-half
            for half in range(2):
                ym_out = ym_outs[layer][half]
                for tb in range(4 * half, 4 * half + 4):
                    u = wp.tile([128, D], F16, name="u_t", tag="u_t", bufs=2)
                    nc.sync.dma_start_transpose(
                        u[:], ym_out[:, 128 * (tb - 4 * half):
                                     128 * (tb - 4 * half) + 128])
                    xm_u, inv_u = _ln_free(nc, wp, u[:], eps_t[:],
                                           name=f"u{tb}")
                    v = wp.tile([128, D], F32, name="v_t", tag="v_t")
                    nc.vector.scalar_tensor_tensor(
                        v[:], xm_u, inv_u[:], x_t16[tb][:],
                        op0=ALU.mult, op1=ALU.add)
                    _ln_free(nc, wp, v[:], eps_t[:],
                             out_f16=x_t16[tb][:], skip_mean=True,
                             name=f"v{tb}")
                    for d in range(DC):
                        nc.sync.dma_start_transpose(
                            x_d16[d][:, 128 * tb:128 * (tb + 1)],
                            x_t16[tb][:, 128 * d:128 * (d + 1)])

        # ------------------------------------------------------- lm head
        for tb in range(TB):
            ps_l = psw(f"ps_lg_{tb}", (128, VOCAB))
            for d in range(DC):
                nc.tensor.matmul(ps_l[:], x_d16[d][:, 128 * tb:128 * (tb + 1)],
                                 lmh_sb[d][:], start=(d == 0),
                                 stop=(d == DC - 1))
            lg_sb = wp.tile([128, VOCAB], F32, name="lg_sb", tag="lg_sb")
            nc.vector.tensor_copy(lg_sb[:], ps_l[:])
            nc.sync.dma_start(out_o[128 * tb:128 * (tb + 1), :], lg_sb[:])

    nc.compile()
    return nc


# ------------------------------------------------------------- host helpers
def _host_tables():
    """cos/sin rope tables in [pair, t] layout, mirroring reference fp32 math."""
    n = np.arange(N, dtype=np.float32)
    q = np.floor(n / 2.0) * 2.0
    freqs = (1.0 / (np.float32(THETA) ** (q / np.float32(N)))
             / np.float32(2.0 * math.pi)).astype(np.float32)
    t = np.arange(T, dtype=np.float32)
    phases = (t[:, None] * freqs[None, :]) % 1.0
    phases = phases * np.float32(2.0 * math.pi)
    cos = np.cos(phases).astype(np.float32)   # [T, N]
    sin = np.sin(phases).astype(np.float32)
    # pair p uses freq of n=2p; table[p, t]
    cos_p = cos[:, 0::2].T.copy()  # [N//2, T]
    sin_p = sin[:, 0::2].T.copy()
    return cos_p, sin_p


def _perm_local():
    """Local latent permutation: position -> (pair index, odd flag)."""
    pos_to_pair = np.empty(NHALF, dtype=np.int64)
    pos_is_odd = np.empty(NHALF, dtype=np.int64)
    for j in range(NJ):
        pr = np.arange(128) + 128 * j
        pos_to_pair[256 * j:256 * j + 128] = pr
        pos_is_odd[256 * j:256 * j + 128] = 0
        pos_to_pair[256 * j + 128:256 * j + 256] = pr
        pos_is_odd[256 * j + 128:256 * j + 256] = 1
    return pos_to_pair, pos_is_odd


_NC_CACHE = {}


def _get_nc():
    if "nc" not in _NC_CACHE:
        _NC_CACHE["nc"] = build_program()
    return _NC_CACHE["nc"]


def prepare_in_maps(idx, embed, encoder, encoder_v, decoder, lm_head):
    idx = np.asarray(idx)
    embed = np.asarray(embed, dtype=np.float32)
    encoder = np.asarray(encoder, dtype=np.float32)
    encoder_v = np.asarray(encoder_v, dtype=np.float32)
    decoder = np.asarray(decoder, dtype=np.float32)
    lm_head = np.asarray(lm_head, dtype=np.float32)

    cos_p, sin_p = _host_tables()
    pos_to_pair, pos_is_odd = _perm_local()

    cmask = (np.arange(128)[:, None] < np.arange(128)[None, :]).astype(np.float16)
    idx32 = idx.reshape(T).astype(np.float32).reshape(T, 1)
    lmh16 = lm_head.astype(np.float16)

    in_maps = []
    for c in range(NCORES):
        h, eta = c // 2, c % 2
        pair_g = NPAIR * eta + pos_to_pair          # global pair index
        n_orig = 2 * pair_g + pos_is_odd            # original n within head
        enc_sh = encoder[h][:, n_orig].astype(np.float16)
        encv_sh = encoder_v[h][:, n_orig].astype(np.float16)
        dec_sh = decoder[h * N + n_orig, :].astype(np.float16)
        cos_sh = cos_p[NPAIR * eta:NPAIR * (eta + 1), :].astype(np.float16)
        sin_sh = sin_p[NPAIR * eta:NPAIR * (eta + 1), :].astype(np.float16)
        cos2 = np.concatenate([cos_sh, sin_sh], axis=1)  # [NPAIR, 2T] = [c|s]
        in_maps.append({
            "idx32": idx32, "embed": embed, "enc_sh": enc_sh,
            "encv_sh": encv_sh, "dec_sh": dec_sh, "lmh": lmh16,
            "cos2_sh": cos2, "cmask": cmask,
        })
    return in_maps


def kernel(idx, embed, encoder, encoder_v, decoder, lm_head):
    in_maps = prepare_in_maps(idx, embed, encoder, encoder_v, decoder,
                              lm_head)
    nc = _get_nc()
    res = bass_utils.run_bass_kernel_spmd(nc, in_maps,
                                          core_ids=list(range(NCORES)))
    _NC_CACHE["last_results"] = res
    logits = np.asarray(res.results[0]["logits"], dtype=np.float32)
    return logits.reshape(1, T, VOCAB)


# revision 19
# speedup vs baseline: 1.6032x; 1.0645x over previous
"""Trainium2 Bass kernel for nn_BDH_6313601925221 (sparse_attention).

Model (reference.py):
  x = LN(embed[idx])                                   (B=1, T=1024, D=256)
  repeat 6 layers (shared weights):
    x_sparse = relu(einsum('btd,hdn->bhtn', x, encoder))   N=8192, NH=4
    QR       = rope(x_sparse)                              interleaved-pair rotation
    scores   = einsum('bhtn,bhsn->bhts', QR, QR) * strict_causal
    yKV      = LN(einsum('bhts,bsd->bhtd', scores, x))
    y_sparse = relu(einsum('bhtd,hdn->bhtn', yKV, encoder_v))
    yMLP     = (x_sparse*y_sparse).transpose -> (T, NH*N) @ decoder
    x        = LN(x + LN(yMLP))
  logits = x @ lm_head

Distribution (8 cores): core c = (head h=c//2, latent-half eta=c%2).
Each core computes the encoder/rope/scores path over its 4096 latent dims
(pairwise AllReduce of partial scores within the head pair), duplicates the
small yKV path, then computes y_sparse/xy/decoder over its latent half for
all tokens; the yMLP partials are AllReduced over all 8 ranks in two
t-half chunks so the collective overlaps the remaining compute.

Key algebraic simplification: the inner LN on yKV can be dropped entirely.
LN is scale-invariant per row, relu is positively homogeneous, and the
whole yKV->y_sparse->xy->decoder path is linear in the per-token scale, so
the per-token 1/std (and the exactly-zero mean, since x rows are zero-mean
LN outputs) cancels inside the tail LN(yMLP). This lets yKV be produced
DIRECTLY in transposed [d, t] layout by the PE (lhsT = x tiles, rhs = score
strips) with no LayerNorm, no transposes.

Scheduling:
  - j-loop software-pipelined: scores matmuls for pair j-1 are emitted
    after the encoder matmuls of pair j, so the PE never waits on the
    relu->rope chain and the HAM clock stays warm.
  - rope runs on wide [128,2048] paired tiles (xs2=[xe|xo] against
    cs2=[c|s], sc2=[s|c]): 3 DVE ops + 1 GpSimd op per pair.
  - phase 3 (y_sparse/xy/decoder) runs in two 512-wide t-half passes;
    the yMLP AllReduce is split per t-half and overlaps the other pass /
    the tail. Score-strip collective bounce is 3 big DMAs per group.
  - decoder weights + lm_head resident in SBUF across all layers.

PSUM budget (8 banks): acc_a/acc_b/acc_c [128,1024] f32 (2 banks each,
bufs=1) carry scores strips, then ym accumulators (acc_a/b) and the yKV^T
accumulators (acc_c); ps_w [128,512]-padded (bufs=2) carries transient
matmul outputs.
"""

import math
import sys

import numpy as np

for _p in ("/opt/trn_rl_repo",):
    if _p not in sys.path:
        sys.path.insert(0, _p)

import concourse.bass as bass
import concourse.mybir as mybir
import concourse.tile as tile
from concourse import bacc
from concourse import bass_utils

# ---------------------------------------------------------------- constants
D = 256
NH = 4
N = 8192
T = 1024
N_LAYER = 6
VOCAB = 256
THETA = 2 ** 16
EPS = 1e-5
NCORES = 8

NHALF = N // 2          # 4096 latent dims per core
NPAIR = NHALF // 2      # 2048 rope pairs per core
NT = NHALF // 128       # 32 local n-tiles of 128
NJ = NT // 2            # 16 pair-blocks (tile 2j = evens, 2j+1 = odds)
TB = T // 128           # 8 token blocks
DC = D // 128           # 2 d-chunks

F16 = mybir.dt.float16
F32 = mybir.dt.float32
I32 = mybir.dt.int32
AX = mybir.AxisListType
ALU = mybir.AluOpType
ACTF = mybir.ActivationFunctionType

ALPHA = 1.0 / 512.0     # yKV pre-scale (overflow headroom; cancels exactly)
BETA = 1.0 / 16.0       # extra scale inside Square so sq fits fp16

# Three score-strip groups, AllReduced separately so each lands just in
# time for the consumer: A in-loop; B unblocks ykvT/phase-3 t-half 0;
# C unblocks t-half 1.  Per group: kb -> (acc tag, column offset).
GROUPS = [
    {"kbs": [0, 1, 2], "lay": {0: ("acc_a", 0), 1: ("acc_b", 0),
                               2: ("acc_c", 0)},
     "rows": {0: 0, 1: 8, 2: 15}, "blocks": 21},
    {"kbs": [3], "lay": {3: ("acc_a", 0)},
     "rows": {3: 0}, "blocks": 5},
    {"kbs": [4, 5, 6, 7], "lay": {4: ("acc_a", 0), 5: ("acc_a", 512),
                                  6: ("acc_b", 0), 7: ("acc_b", 512)},
     "rows": {4: 0, 5: 4, 6: 7, 7: 9}, "blocks": 10},
]


def _ln_free(nc, pool, x_ap, eps_ap, out_f32=None, out_f16=None,
             skip_mean=False, n=None, name=""):
    """LayerNorm along the free dim of a [128, n] tile (per-partition stats)."""
    n = n if n is not None else x_ap.shape[-1]
    inv_n = 1.0 / n
    sq = pool.tile([128, n], F16, name=f"lnsq{name}", tag="lnsq")
    ssq = pool.tile([128, 1], F32, name=f"lnssq{name}", tag="lnssq")
    std = pool.tile([128, 1], F32, name=f"lnstd{name}", tag="lnstd")
    inv = pool.tile([128, 1], F32, name=f"lninv{name}", tag="lninv")
    if skip_mean:
        xm = x_ap
    else:
        mu = pool.tile([128, 1], F32, name=f"lnmu{name}", tag="lnmu")
        xm_t = pool.tile([128, n], F32, name=f"lnxm{name}", tag="lnxm")
        nc.vector.tensor_reduce(mu[:], x_ap, axis=AX.X, op=ALU.add)
        nc.scalar.mul(mu[:], mu[:], inv_n)
        nc.vector.tensor_scalar_sub(xm_t[:], x_ap, mu[:])
        xm = xm_t[:]
    nc.scalar.activation(sq[:], xm, ACTF.Square, accum_out=ssq[:])
    nc.scalar.activation(std[:], ssq[:], ACTF.Sqrt, bias=eps_ap, scale=inv_n)
    nc.vector.reciprocal(inv[:], std[:])
    if out_f32 is not None:
        nc.vector.tensor_scalar_mul(out_f32, xm, inv[:])
    if out_f16 is not None:
        nc.scalar.activation(out_f16, xm, ACTF.Copy, scale=inv[:])
    return xm, inv


def build_program(n_layer=N_LAYER):
    nc = bacc.Bacc("TRN2", target_bir_lowering=False, debug=False,
                   num_devices=NCORES)

    # ------------------------------------------------------------- I/O decl
    idx_i = nc.dram_tensor("idx32", [T, 1], F32, kind="ExternalInput")
    embed_i = nc.dram_tensor("embed", [VOCAB, D], F32, kind="ExternalInput")
    enc_i = nc.dram_tensor("enc_sh", [D, NHALF], F16, kind="ExternalInput")
    encv_i = nc.dram_tensor("encv_sh", [D, NHALF], F16, kind="ExternalInput")
    dec_i = nc.dram_tensor("dec_sh", [NHALF, D], F16, kind="ExternalInput")
    lmh_i = nc.dram_tensor("lmh", [D, VOCAB], F16, kind="ExternalInput")
    cos2_i = nc.dram_tensor("cos2_sh", [NPAIR, 2 * T], F16,
                            kind="ExternalInput")
    cmask_i = nc.dram_tensor("cmask", [128, 128], F16, kind="ExternalInput")
    out_o = nc.dram_tensor("logits", [T, VOCAB], F32, kind="ExternalOutput")

    pair_groups = [[2 * h, 2 * h + 1] for h in range(NH)]
    all_group = [list(range(NCORES))]

    with tile.TileContext(nc) as tc:
      with (
        tc.tile_pool(name="persist", bufs=1) as pp,
        tc.tile_pool(name="work", bufs=2) as wp,
        tc.tile_pool(name="psW", bufs=2, space="PSUM") as psW,
        tc.tile_pool(name="psAcc", bufs=1, space="PSUM") as psAcc,
        tc.tile_pool(name="dram", bufs=1, space="DRAM") as dp,
      ):
        # ------------------------------------------------- persistent SBUF
        enc_sb = [pp.tile([128, NHALF], F16, name=f"enc{d}", tag=f"enc{d}")
                  for d in range(DC)]
        encv_sb = [pp.tile([128, NHALF], F16, name=f"encv{d}", tag=f"encv{d}")
                   for d in range(DC)]
        dec_sb = [pp.tile([128, D], F16, name=f"dec{i}", tag=f"dec{i}")
                  for i in range(NT)]
        QR = [pp.tile([128, T], F16, name=f"qr{i}", tag=f"qr{i}")
              for i in range(NT)]
        # score strips S^T[kb]: [128 s, (TB-kb)*128 q] fp16, diag-masked
        ST = [pp.tile([128, (TB - kb) * 128], F16, name=f"st{kb}",
                      tag=f"st{kb}") for kb in range(TB)]
        x_t16 = [pp.tile([128, D], F16, name=f"xt16_{i}", tag=f"xt16_{i}")
                 for i in range(TB)]
        x_d16 = [pp.tile([128, T], F16, name=f"xd16_{i}", tag=f"xd16_{i}")
                 for i in range(DC)]
        ykvT = [pp.tile([128, T], F16, name=f"ykvT{i}", tag=f"ykvT{i}")
                for i in range(DC)]
        cmask = pp.tile([128, 128], F16, name="cmaskt", tag="cmaskt")
        eps_t = pp.tile([128, 1], F32, name="eps_t", tag="eps_t")
        ones_t = pp.tile([128, 1], F16, name="ones_t", tag="ones_t")
        eps2_t = pp.tile([1, 1], F32, name="eps2_t", tag="eps2_t")
        lmh_sb = [pp.tile([128, VOCAB], F16, name=f"lmh{d}", tag=f"lmh{d}")
                  for d in range(DC)]

        # ---------------------------------------------------- DRAM buffers
        xs_spill = dp.tile([NHALF, T], F16, name="xs_spill")
        sc_ins = [dp.tile([g["blocks"] * 128, 128], F16, name=f"sc_in{gi}",
                          tag=f"sc_in{gi}")
                  for gi, g in enumerate(GROUPS)]
        sc_outs = [dp.tile([g["blocks"] * 128, 128], F16, name=f"sc_out{gi}",
                           tag=f"sc_out{gi}")
                   for gi, g in enumerate(GROUPS)]
        ym_ins = [dp.tile([D, 512], F16, name=f"ym_in{h}", tag=f"ym_in{h}")
                  for h in range(2)]
        ym_outs = [[dp.tile([D, 512], F16, name=f"ym_out{l}_{h}",
                            tag=f"ym_out{l}_{h}", addr_space="Shared")
                    for h in range(2)] for l in range(n_layer)]

        def psw(name, shape=(128, 512), dtype=F32):
            return psW.tile(list(shape), dtype, name=name, tag="ps_w",
                            padded_shape=[128, 512])

        # ------------------------------------------------------ load consts
        nc.gpsimd.memset(eps_t[:], EPS)
        nc.gpsimd.memset(ones_t[:], 1.0)
        nc.gpsimd.memset(eps2_t[:], EPS * ALPHA * ALPHA)
        nc.sync.dma_start(cmask[:], cmask_i[:, :])
        for d in range(DC):
            nc.sync.dma_start(enc_sb[d][:], enc_i[128 * d:128 * (d + 1), :])
            nc.sync.dma_start(encv_sb[d][:], encv_i[128 * d:128 * (d + 1), :])
            nc.sync.dma_start(lmh_sb[d][:], lmh_i[128 * d:128 * (d + 1), :])
        for i in range(NT):
            nc.scalar.dma_start(dec_sb[i][:], dec_i[128 * i:128 * (i + 1), :])

        # ------------------------------------------------------- embedding
        # E_n = LN(embed) per vocab row; x0 = onehot(idx) @ E_n
        with tc.tile_pool(name="embed", bufs=1) as ep:
            E_n = [ep.tile([128, D], F16, name=f"en{v}", tag=f"en{v}")
                   for v in range(DC)]
            for v in range(DC):
                emb_raw = ep.tile([128, D], F32, name=f"emb_raw{v}",
                                  tag=f"emb_raw{v}")
                nc.sync.dma_start(emb_raw[:], embed_i[128 * v:128 * (v + 1), :])
                _ln_free(nc, wp, emb_raw[:], eps_t[:], out_f16=E_n[v][:],
                         name=f"emb{v}")

            iota_i32 = ep.tile([128, VOCAB], I32, name="iota_i32",
                               tag="iota_i32")
            nc.gpsimd.iota(iota_i32[:], pattern=[[1, VOCAB]], base=0,
                           channel_multiplier=0)
            iota_t = ep.tile([128, VOCAB], F32, name="iota_t", tag="iota_t")
            nc.vector.tensor_copy(iota_t[:], iota_i32[:])
            OHT = [ep.tile([128, T], F16, name=f"oht{v}", tag=f"oht{v}")
                   for v in range(DC)]
            for tb in range(TB):
                idx_col = wp.tile([128, 1], F32, name="idx_col", tag="idx_col")
                nc.sync.dma_start(idx_col[:], idx_i[128 * tb:128 * (tb + 1), :])
                oh_tm = wp.tile([128, VOCAB], F16, name="oh_tm", tag="oh_tm",
                                bufs=3)
                nc.vector.tensor_scalar(oh_tm[:], iota_t[:], idx_col[:], None,
                                        op0=ALU.is_equal)
                for v in range(DC):
                    nc.sync.dma_start_transpose(
                        OHT[v][:, 128 * tb:128 * (tb + 1)],
                        oh_tm[:, 128 * v:128 * (v + 1)])

            for tb in range(TB):
                ps_x = psw(f"ps_x0_{tb}", (128, D))
                for v in range(DC):
                    nc.tensor.matmul(ps_x[:],
                                     OHT[v][:, 128 * tb:128 * (tb + 1)],
                                     E_n[v][:], start=(v == 0),
                                     stop=(v == DC - 1))
                nc.scalar.copy(x_t16[tb][:], ps_x[:])
            for d in range(DC):
                for th in range(2):
                    ps_xd = psw(f"ps_xd_{d}_{th}")
                    for v in range(DC):
                        nc.tensor.matmul(
                            ps_xd[:], E_n[v][:, 128 * d:128 * (d + 1)],
                            OHT[v][:, 512 * th:512 * (th + 1)],
                            start=(v == 0), stop=(v == DC - 1))
                    nc.scalar.copy(x_d16[d][:, 512 * th:512 * (th + 1)],
                                   ps_xd[:])

        # ============================================================ layers
        for layer in range(n_layer):
            def sc_ap(kb, lay, grp_acc):
                tag, off = lay[kb]
                w = (TB - kb) * 128
                return grp_acc[tag][:, off:off + w]

            def scores_block(nt, g, grp_acc):
                for kb in g["kbs"]:
                    dst = sc_ap(kb, g["lay"], grp_acc)
                    w = (TB - kb) * 128
                    for nn in range(0, w, 512):
                        nw = min(512, w - nn)
                        nc.tensor.matmul(
                            dst[:, nn:nn + nw],
                            QR[nt][:, 128 * kb:128 * (kb + 1)],
                            QR[nt][:, 128 * kb + nn:128 * kb + nn + nw],
                            start=(nt == 0), stop=(nt == NT - 1))

            def spill_group(gi, grp_acc):
                g = GROUPS[gi]
                for kb in g["kbs"]:
                    w = (TB - kb) * 128
                    s_sb = wp.tile([128, w], F16, name=f"s_sb{kb}",
                                   tag="s_sb", padded_shape=[128, 1024])
                    nc.scalar.copy(s_sb[:], sc_ap(kb, g["lay"], grp_acc))
                    dst = sc_ins[gi][:].rearrange("(b p) n -> p b n", p=128)
                    nc.sync.dma_start(
                        dst[:, g["rows"][kb]:g["rows"][kb] + (TB - kb), :],
                        s_sb[:].rearrange("p (b n) -> p b n", n=128))
                nc.gpsimd.collective_compute(
                    "AllReduce", ALU.add, replica_groups=pair_groups,
                    ins=[sc_ins[gi].opt()], outs=[sc_outs[gi].opt()])

            def load_groupg(gi):
                g = GROUPS[gi]
                for kb in g["kbs"]:
                    src = sc_outs[gi][:].rearrange("(b p) n -> p b n", p=128)
                    nc.sync.dma_start(
                        ST[kb][:].rearrange("p (b n) -> p b n", n=128),
                        src[:, g["rows"][kb]:g["rows"][kb] + (TB - kb), :])
                    nc.vector.tensor_mul(ST[kb][:, 0:128], ST[kb][:, 0:128],
                                         cmask[:])

            # ---------------- phase 1a: x_sparse + rope + scores group A
            accA = {t: psAcc.tile([128, 1024], F32, name=f"{t}_sA_{layer}",
                                  tag=t) for t in ("acc_a", "acc_b", "acc_c")}
            for j in range(NJ):
                cs2 = wp.tile([128, 2 * T], F16, name="cs2", tag="cs2",
                              bufs=2)
                nc.sync.dma_start(cs2[:], cos2_i[128 * j:128 * (j + 1), :])
                # xs2 = [xs_even | xs_odd] for this pair; t-half 0 encoder
                # matmuls first so a new layer can begin before the second
                # ym AllReduce has fully landed.
                xs2 = wp.tile([128, 2 * T], F16, name="xs2", tag="xs2",
                              bufs=3)
                for th in range(2):
                    for par in range(2):
                        nt = 2 * j + par
                        ps_e = psw(f"ps_enc_{layer}_{nt}_{th}")
                        for d in range(DC):
                            nc.tensor.matmul(
                                ps_e[:],
                                enc_sb[d][:, 128 * nt:128 * (nt + 1)],
                                x_d16[d][:, 512 * th:512 * (th + 1)],
                                start=(d == 0), stop=(d == DC - 1))
                        nc.scalar.activation(
                            xs2[:, 1024 * par + 512 * th:
                                1024 * par + 512 * (th + 1)],
                            ps_e[:], ACTF.Relu)
                nc.scalar.dma_start(
                    xs_spill[256 * j:256 * (j + 1), :].rearrange(
                        "(b p) n -> p b n", p=128),
                    xs2[:].rearrange("p (b n) -> p b n", n=T))
                # rope: cs2 = [c|s], xs2 = [xe|xo]
                #   m1 = xs2*cs2 = [xe*c | xo*s];  qe = m1_lo - m1_hi
                #   qo = xo*c + xe*s
                qe, qo = QR[2 * j], QR[2 * j + 1]
                m1 = wp.tile([128, 2 * T], F16, name="m1", tag="rope_m",
                             bufs=2)
                nc.vector.tensor_mul(m1[:], xs2[:], cs2[:])
                nc.vector.tensor_sub(qe[:], m1[:, 0:T], m1[:, T:2 * T])
                m2 = wp.tile([128, 2 * T], F16, name="m2", tag="rope_m",
                             bufs=2)
                nc.vector.tensor_mul(m2[:, 0:T], xs2[:, T:2 * T],
                                     cs2[:, 0:T])
                nc.vector.tensor_mul(m2[:, T:2 * T], xs2[:, 0:T],
                                     cs2[:, T:2 * T])
                nc.gpsimd.tensor_add(qo[:], m2[:, 0:T], m2[:, T:2 * T])
                # scores for pair j-2 (depth-2 software pipeline: PE never
                # waits on the relu/rope chain)
                if j >= 2:
                    scores_block(2 * (j - 2), GROUPS[0], accA)
                    scores_block(2 * (j - 2) + 1, GROUPS[0], accA)
            for p in (NJ - 2, NJ - 1):
                scores_block(2 * p, GROUPS[0], accA)
                scores_block(2 * p + 1, GROUPS[0], accA)
            spill_group(0, accA)

            # ---------------- phase 1b: scores group B (kb3), then C
            accB = {"acc_a": psAcc.tile([128, 1024], F32,
                                        name=f"acc_a_sB_{layer}",
                                        tag="acc_a")}
            for nt in range(NT):
                scores_block(nt, GROUPS[1], accB)
            spill_group(1, accB)

            accC = {t: psAcc.tile([128, 1024], F32, name=f"{t}_sC_{layer}",
                                  tag=t) for t in ("acc_a", "acc_b")}
            for nt in range(NT):
                scores_block(nt, GROUPS[2], accC)
            spill_group(2, accC)

            load_groupg(0)
            load_groupg(1)

            # ym accumulators (tags a/b become free once C is spilled)
            ym_acc = {}
            for dh in range(DC):
                t = ("acc_a", "acc_b")[dh]
                ym_acc[dh] = psAcc.tile([128, 1024], F32,
                                        name=f"{t}_ym_{layer}", tag=t)

            def ykv_half(h):
                # ykvT[dc][:, 512h:512h+512] = sum_s x[s,dc]*ST[s, half h];
                # q-column c needs strips kb <= c//128.
                yh = psAcc.tile([128, 1024], F32, name=f"ykv_ps_{layer}_{h}",
                                tag="acc_c")
                kmax = 3 if h == 0 else TB - 1
                for dc in range(DC):
                    for kb in range(kmax + 1):
                        a = max(512 * h, 128 * kb)
                        b = 512 * (h + 1)
                        nc.tensor.matmul(
                            yh[:, 512 * dc + a - 512 * h:512 * dc + 512],
                            x_t16[kb][:, 128 * dc:128 * (dc + 1)],
                            ST[kb][:, a - 128 * kb:b - 128 * kb],
                            start=(kb == 0), stop=(kb == kmax))
                for dc in range(DC):
                    nc.scalar.mul(ykvT[dc][:, 512 * h:512 * (h + 1)],
                                  yh[:, 512 * dc:512 * dc + 512], ALPHA)

            def stats_half(h):
                # per-(head,token) 1/std of yKV over this t-half (deferred
                # inner LayerNorm; rows zero-mean so var = E[y^2]); applied
                # to the ym partials pre-AllReduce -- exact, incl. eps.
                ssq_ps = psW.tile([1, 512], F32, name=f"ssq_{layer}_{h}",
                                  tag="ps_w", padded_shape=[128, 512])
                for dc in range(DC):
                    sqt = wp.tile([128, 512], F16, name="sqt", tag="sqt",
                                  bufs=2)
                    nc.scalar.activation(sqt[:],
                                         ykvT[dc][:, 512 * h:512 * (h + 1)],
                                         ACTF.Square, scale=BETA)
                    nc.tensor.matmul(ssq_ps[:], ones_t[:], sqt[:],
                                     start=(dc == 0), stop=(dc == DC - 1))
                std_row = wp.tile([1, 512], F32, name="std_row",
                                  tag="std_row", bufs=2)
                nc.scalar.activation(std_row[:], ssq_ps[:], ACTF.Sqrt,
                                     bias=eps2_t[:],
                                     scale=1.0 / (D * BETA * BETA))
                inv_row = wp.tile([1, 512], F32, name="inv_row",
                                  tag="inv_row", bufs=2)
                nc.vector.reciprocal(inv_row[:], std_row[:])
                ib = wp.tile([128, 512], F32, name=f"inv_b{h}", tag="inv_b",
                             bufs=2)
                nc.gpsimd.partition_broadcast(ib[:], inv_row[:])
                return ib

            def emit_ym(nt, th, xy):
                for dh in range(DC):
                    nc.tensor.matmul(
                        ym_acc[dh][:, 512 * th:512 * (th + 1)],
                        dec_sb[nt][:, 128 * dh:128 * (dh + 1)],
                        xy[:], start=(nt == 0), stop=(nt == NT - 1))

            def phase3_pass(th):
                prev = None
                for nt in range(NT):
                    ps_v = psw(f"ps_ysp_{layer}_{nt}_{th}")
                    for d in range(DC):
                        nc.tensor.matmul(
                            ps_v[:], encv_sb[d][:, 128 * nt:128 * (nt + 1)],
                            ykvT[d][:, 512 * th:512 * (th + 1)],
                            start=(d == 0), stop=(d == DC - 1))
                    xs_c = wp.tile([128, 512], F16, name="xs_c", tag="xs_c",
                                   bufs=3)
                    nc.scalar.dma_start(
                        xs_c[:],
                        xs_spill[128 * nt:128 * (nt + 1),
                                 512 * th:512 * (th + 1)])
                    xy = wp.tile([128, 512], F16, name="xy", tag="xy", bufs=3)
                    # xy = relu(ys) * xs  (fused)
                    nc.vector.scalar_tensor_tensor(
                        xy[:], ps_v[:], 0.0, xs_c[:],
                        op0=ALU.max, op1=ALU.mult)
                    if prev is not None:
                        emit_ym(*prev)
                    prev = (nt, th, xy)
                emit_ym(*prev)

            def ym_reduce(th, ib):
                for dh in range(DC):
                    ym_sb = wp.tile([128, 512], F16, name=f"ym_sb{th}",
                                    tag="ym_sb", bufs=2)
                    nc.vector.tensor_mul(ym_sb[:],
                                         ym_acc[dh][:, 512 * th:
                                                    512 * (th + 1)],
                                         ib[:])
                    nc.sync.dma_start(ym_ins[th][128 * dh:128 * (dh + 1), :],
                                      ym_sb[:])
                nc.gpsimd.collective_compute(
                    "AllReduce", ALU.add, replica_groups=all_group,
                    ins=[ym_ins[th].opt()], outs=[ym_outs[layer][th].opt()])

            # t-half 0 needs only strips kb<=3 (groups A+B)
            ykv_half(0)
            ib0 = stats_half(0)
            phase3_pass(0)
            ym_reduce(0, ib0)

            # t-half 1 needs all strips (group C lands during pass 0)
            load_groupg(2)
            ykv_half(1)
            ib1 = stats_half(1)
            phase3_pass(1)
            ym_reduce(1, ib1)

            # ---------------- tail: x = LN(x + LN(yMLP)), per t-half
            for half in range(2):
                ym_out = ym_outs[layer][half]
                for tb in range(4 * half, 4 * half + 4):
                    u = wp.tile([128, D], F16, name="u_t", tag="u_t", bufs=2)
                    nc.sync.dma_start_transpose(
                        u[:], ym_out[:, 128 * (tb - 4 * half):
                                     128 * (tb - 4 * half) + 128])
                    xm_u, inv_u = _ln_free(nc, wp, u[:], eps_t[:],
                                           name=f"u{tb}")
                    v = wp.tile([128, D], F32, name="v_t", tag="v_t")
                    nc.vector.scalar_tensor_tensor(
                        v[:], xm_u, inv_u[:], x_t16[tb][:],
                        op0=ALU.mult, op1=ALU.add)
                    _ln_free(nc, wp, v[:], eps_t[:],
                             out_f16=x_t16[tb][:], skip_mean=True,
                             name=f"v{tb}")
                    for d in range(DC):
                        nc.sync.dma_start_transpose(
                            x_d16[d][:, 128 * tb:128 * (tb + 1)],
                            x_t16[tb][:, 128 * d:128 * (d + 1)])

        # ------------------------------------------------------- lm head
        for tb in range(TB):
            ps_l = psw(f"ps_lg_{tb}", (128, VOCAB))
            for d in range(DC):
                nc.tensor.matmul(ps_l[:], x_d16[d][:, 128 * tb:128 * (tb + 1)],
                                 lmh_sb[d][:], start=(d == 0),
                                 stop=(d == DC - 1))
            lg_sb = wp.tile([128, VOCAB], F32, name="lg_sb", tag="lg_sb")
            nc.vector.tensor_copy(lg_sb[:], ps_l[:])
            nc.sync.dma_start(out_o[128 * tb:128 * (tb + 1), :], lg_sb[:])

    nc.compile()
    return nc


# ------------------------------------------------------------- host helpers
def _host_tables():
    """cos/sin rope tables in [pair, t] layout, mirroring reference fp32 math."""
    n = np.arange(N, dtype=np.float32)
    q = np.floor(n / 2.0) * 2.0
    freqs = (1.0 / (np.float32(THETA) ** (q / np.float32(N)))
             / np.float32(2.0 * math.pi)).astype(np.float32)
    t = np.arange(T, dtype=np.float32)
    phases = (t[:, None] * freqs[None, :]) % 1.0
    phases = phases * np.float32(2.0 * math.pi)
    cos = np.cos(phases).astype(np.float32)   # [T, N]
    sin = np.sin(phases).astype(np.float32)
    # pair p uses freq of n=2p; table[p, t]
    cos_p = cos[:, 0::2].T.copy()  # [N//2, T]
    sin_p = sin[:, 0::2].T.copy()
    return cos_p, sin_p


def _perm_local():
    """Local latent permutation: position -> (pair index, odd flag)."""
    pos_to_pair = np.empty(NHALF, dtype=np.int64)
    pos_is_odd = np.empty(NHALF, dtype=np.int64)
    for j in range(NJ):
        pr = np.arange(128) + 128 * j
        pos_to_pair[256 * j:256 * j + 128] = pr
        pos_is_odd[256 * j:256 * j + 128] = 0
        pos_to_pair[256 * j + 128:256 * j + 256] = pr
        pos_is_odd[256 * j + 128:256 * j + 256] = 1
    return pos_to_pair, pos_is_odd


_NC_CACHE = {}


def _get_nc():
    if "nc" not in _NC_CACHE:
        _NC_CACHE["nc"] = build_program()
    return _NC_CACHE["nc"]


def prepare_in_maps(idx, embed, encoder, encoder_v, decoder, lm_head):
    idx = np.asarray(idx)
    embed = np.asarray(embed, dtype=np.float32)
    encoder = np.asarray(encoder, dtype=np.float32)
    encoder_v = np.asarray(encoder_v, dtype=np.float32)
    decoder = np.asarray(decoder, dtype=np.float32)
    lm_head = np.asarray(lm_head, dtype=np.float32)

    cos_p, sin_p = _host_tables()
    pos_to_pair, pos_is_odd = _perm_local()

    cmask = (np.arange(128)[:, None] < np.arange(128)[None, :]).astype(np.float16)
    idx32 = idx.reshape(T).astype(np.float32).reshape(T, 1)
    lmh16 = lm_head.astype(np.float16)

    in_maps = []
    for c in range(NCORES):
        h, eta = c // 2, c % 2
        pair_g = NPAIR * eta + pos_to_pair          # global pair index
        n_orig = 2 * pair_g + pos_is_odd            # original n within head
        enc_sh = encoder[h][:, n_orig].astype(np.float16)
        encv_sh = encoder_v[h][:, n_orig].astype(np.float16)
        dec_sh = decoder[h * N + n_orig, :].astype(np.float16)
        cos_sh = cos_p[NPAIR * eta:NPAIR * (eta + 1), :].astype(np.float16)
        sin_sh = sin_p[NPAIR * eta:NPAIR * (eta + 1), :].astype(np.float16)
        cos2 = np.concatenate([cos_sh, sin_sh], axis=1)  # [NPAIR, 2T] = [c|s]
        in_maps.append({
            "idx32": idx32, "embed": embed, "enc_sh": enc_sh,
            "encv_sh": encv_sh, "dec_sh": dec_sh, "lmh": lmh16,
            "cos2_sh": cos2, "cmask": cmask,
        })
    return in_maps


def kernel(idx, embed, encoder, encoder_v, decoder, lm_head):
    in_maps = prepare_in_maps(idx, embed, encoder, encoder_v, decoder,
                              lm_head)
    nc = _get_nc()
    res = bass_utils.run_bass_kernel_spmd(nc, in_maps,
                                          core_ids=list(range(NCORES)))
    _NC_CACHE["last_results"] = res
    logits = np.asarray(res.results[0]["logits"], dtype=np.float32)
    return logits.reshape(1, T, VOCAB)
